# revision 10
# baseline (speedup 1.0000x reference)
"""Trainium2 Bass kernel for additive attention (nn_Attention).

Reference computation (per batch b):
    att_h  = h @ W.T + b_h2att                      [B, ATTH]
    dot    = tanh(p_att_feats + att_h[:, None, :])  [B, S, ATTH]
    scores = dot @ w_alpha[0] (+ b_alpha)           [B, S]
    weight = softmax(scores, axis=1)
    out    = weight @ att_feats                     [B, RNN]

Sharding: data-parallel over batch, 32 batches per core x 8 cores.

All inputs are converted to bf16 on the host (DMA is the roofline) and
W/h are host-pre-transposed. Per-core layout: (batch, S) flattened to
G = 32*196 = 6272 rows = exactly 49 tiles of 128 partitions.

z = p + att_h[batch(row)] is computed two ways to balance engines:
  - "boundary" tiles (rows span 2 batches, 31 of 49): TensorE streams p
    into PSUM via an identity matmul, then a 0/1 mask (bsel) matmul adds
    the right batch's att_h row.
  - "clean" tiles (single batch, 18 of 49): DVE adds a prebuilt
    broadcast tile zbc[t] (att_h[b] replicated to 128 partitions, built
    once at setup by TensorE) to p in SBUF.
dot = tanh(z) on ScalarE (bf16 out, one call per tile pair).
scores: one fused custom-DVE TENSOR_TENSOR_REDUCE per tile
    (prod = dot * w_alpha discarded, accum = sum -> fp32 scores col).
e = exp(scores) (softmax shift bounded: |scores| <~ 20, b_alpha cancels
    in softmax and is dropped).
lhsT[p, b] = e[p] * (batch(p)==b) on DVE (bf16), then TensorE:
    att_res += lhsT.T @ att_tile ; sumexp += lhsT.T @ ones.
Final: out = att_res * (1/sumexp) fused into the PSUM->SBUF copy.
"""

import numpy as np
import ml_dtypes

import concourse.bass as bass
import concourse.tile as tile
from concourse import bacc, mybir
from concourse.bass_utils import run_bass_kernel_spmd
from concourse.dve_ops import TENSOR_TENSOR_REDUCE

F32 = mybir.dt.float32
BF16 = mybir.dt.bfloat16
AF = mybir.ActivationFunctionType
ALU = mybir.AluOpType
BF = ml_dtypes.bfloat16

B, S, RNN, ATTH = 256, 196, 1024, 512
NCORES = 8
BSH = B // NCORES            # 32 batches per core
G = BSH * S                  # 6272 rows per core
NT = G // 128                # 49 tiles
assert NT * 128 == G
GROUPS = [4, 5, 6, 6, 6, 6, 6, 6, 4]  # tiles per DMA group
assert sum(GROUPS) == NT
GSTART = [sum(GROUPS[:i]) for i in range(len(GROUPS))]

# tiles whose 128 rows sit inside a single batch
CLEAN = {t for t in range(NT)
         if (128 * t) // S == (128 * t + 127) // S}


def _pair_up(ts):
    out = []
    for i in range(0, len(ts), 2):
        out.append((ts[i], ts[i + 1] if i + 1 < len(ts) else None))
    return out


_cached = {}


def _batch_of_row(g):
    return g // S


def _rep_groups(repeats):
    for r in range(repeats):
        for g in range(len(GROUPS)):
            yield r, g


def build_nc(repeats=1):
    nc = bacc.Bacc("TRN2", target_bir_lowering=False, debug=False,
                   enable_asserts=True, num_devices=NCORES)

    hT_d = nc.dram_tensor("hT", [128, 8 * BSH], BF16, kind="ExternalInput")
    att_d = nc.dram_tensor("att", [G, RNN], BF16, kind="ExternalInput")
    p_d = nc.dram_tensor("p_att", [G, ATTH], BF16, kind="ExternalInput")
    wT_d = nc.dram_tensor("wT", [128, 8 * ATTH], BF16, kind="ExternalInput")
    bias_d = nc.dram_tensor("b_h2att", [1, ATTH], BF16, kind="ExternalInput")
    walpha_d = nc.dram_tensor("w_alpha", [1, ATTH], BF16,
                              kind="ExternalInput")
    out_d = nc.dram_tensor("out", [BSH, RNN], F32, kind="ExternalOutput")

    # --- host-side constants, embedded in the NEFF ---
    ident_np = np.eye(128, dtype=np.float32).astype(BF)
    ones_np = np.ones((128, 128), dtype=np.float32).astype(BF)
    # maskT[p, t, b] = 1 if batch(128t + p) == b
    maskT_np = np.zeros((128, NT, BSH), dtype=np.float32)
    for t in range(NT):
        for p in range(128):
            bb = _batch_of_row(128 * t + p)
            maskT_np[p, t, bb] = 1.0
    # bsel[b, t, p]: one-hot selector, bsel.T @ att_h broadcasts per-row att_h
    bsel_np = np.ascontiguousarray(maskT_np.transpose(2, 1, 0)).astype(BF)
    maskT_np = maskT_np.astype(BF)

    ident_c = nc.inline_tensor(ident_np, "c_ident")
    ones_c = nc.inline_tensor(ones_np, "c_ones")
    bsel_c = nc.inline_tensor(bsel_np.reshape(BSH, NT * 128), "c_bsel")
    maskT_c = nc.inline_tensor(maskT_np.reshape(128, NT * BSH), "c_maskT")

    # per-group clean/boundary pairings
    g_clean = {}
    g_bnd = {}
    for g in range(len(GROUPS)):
        lo, hi = GSTART[g], GSTART[g] + GROUPS[g]
        g_clean[g] = _pair_up([t for t in range(lo, hi) if t in CLEAN])
        g_bnd[g] = _pair_up([t for t in range(lo, hi) if t not in CLEAN])

    with tile.TileContext(nc) as tc:
        import contextlib
        ctx = contextlib.ExitStack()
        with ctx:
            consts = ctx.enter_context(tc.tile_pool(name="consts", bufs=1))
            work = ctx.enter_context(tc.tile_pool(name="work", bufs=1))
            zbc_pool = ctx.enter_context(tc.tile_pool(name="zbc", bufs=1))
            p_pool = ctx.enter_context(tc.tile_pool(name="p_pool", bufs=2))
            a_pool = ctx.enter_context(tc.tile_pool(name="a_pool", bufs=4))
            setup_sb_cm = tc.tile_pool(name="setup_sb", bufs=1)
            setup_sb = setup_sb_cm.__enter__()
            res_pool = ctx.enter_context(
                tc.tile_pool(name="respsum", bufs=1, space="PSUM"))
            setup_ps_cm = tc.tile_pool(name="setupps", bufs=1, space="PSUM")
            setup_ps = setup_ps_cm.__enter__()

            # ---- load constants and small inputs ----
            ident_sb = consts.tile([128, 128], BF16)
            nc.scalar.dma_start(out=ident_sb[:], in_=ident_c[:])
            hT_sb = setup_sb.tile([128, 8 * BSH], BF16)
            nc.gpsimd.dma_start(out=hT_sb[:], in_=hT_d[:])
            wT_sb = setup_sb.tile([128, 8 * ATTH], BF16)
            nc.gpsimd.dma_start(out=wT_sb[:], in_=wT_d[:])
            ones_sb = consts.tile([128, 128], BF16)
            nc.scalar.dma_start(out=ones_sb[:], in_=ones_c[:])
            onesr_sb = consts.tile([128, 2], BF16)
            nc.scalar.dma_start(out=onesr_sb[:], in_=ones_c[:, 0:2])
            bias_sb = setup_sb.tile([1, ATTH], BF16)
            nc.scalar.dma_start(out=bias_sb[:], in_=bias_d[:])
            walpha_sb = setup_sb.tile([1, ATTH], BF16)
            nc.scalar.dma_start(out=walpha_sb[:], in_=walpha_d[:])
            bsel_sb = consts.tile([BSH, NT * 128], BF16)
            nc.scalar.dma_start(out=bsel_sb[:], in_=bsel_c[:])
            maskT_sb = consts.tile([128, NT * BSH], BF16)
            nc.scalar.dma_start(out=maskT_sb[:], in_=maskT_c[:])

            # ---- att_h = h @ W.T + bias via pre-transposed hT/wT ----
            ah_ps = setup_ps.tile([BSH, ATTH], F32, tag="ahps")
            for rc in range(8):
                nc.tensor.matmul(
                    ah_ps[:],
                    lhsT=hT_sb[:, rc * BSH:(rc + 1) * BSH],
                    rhs=wT_sb[:, rc * ATTH:(rc + 1) * ATTH],
                    start=(rc == 0), stop=False)
            nc.tensor.matmul(
                ah_ps[:], lhsT=ones_sb[0:1, 0:BSH], rhs=bias_sb[0:1, :],
                start=False, stop=True)
            atth_sb = work.tile([BSH, ATTH], BF16)
            nc.vector.tensor_copy(atth_sb[:], ah_ps[:])

            # ---- broadcast w_alpha to all 128 partitions (bf16) ----
            wb_ps = setup_ps.tile([128, ATTH], F32, tag="wbps")
            nc.tensor.matmul(wb_ps[:], lhsT=ones_sb[0:1, 0:128],
                             rhs=walpha_sb[0:1, :], start=True, stop=True)
            wb_sb = work.tile([128, ATTH], BF16)
            nc.vector.tensor_copy(wb_sb[:], wb_ps[:])

            # ---- zbc[t] = att_h[batch(t)] broadcast to 128 partitions,
            # for clean tiles, built once by TensorE + copied to SBUF ----
            zbc_slice = {}
            ci = 0
            for g in range(len(GROUPS)):
                for t0, t1 in g_clean[g]:
                    n_z = 1024 if t1 is not None else 512
                    zb_ps = setup_ps.tile([128, 1024], F32, tag="zb")
                    zbc_t = zbc_pool.tile([128, 1024], BF16, tag=f"zbc{ci}",
                                          name=f"zbc{ci}")
                    for i, t in enumerate((t0, t1)):
                        if t is None:
                            continue
                        nc.tensor.matmul(
                            zb_ps[:, i * 512:(i + 1) * 512],
                            lhsT=bsel_sb[:, t * 128:(t + 1) * 128],
                            rhs=atth_sb[:], start=True, stop=True)
                        zbc_slice[t] = zbc_t[:, i * 512:(i + 1) * 512]
                    if ci % 2 == 0:
                        nc.vector.tensor_copy(zbc_t[:, 0:n_z],
                                              zb_ps[:, 0:n_z])
                    else:
                        nc.scalar.activation(zbc_t[:, 0:n_z],
                                             zb_ps[:, 0:n_z], AF.Copy)
                    ci += 1
            setup_sb_cm.__exit__(None, None, None)
            setup_ps_cm.__exit__(None, None, None)

            zp_pool = ctx.enter_context(
                tc.tile_pool(name="zpsum", bufs=2, space="PSUM"))
            zsb_pool = ctx.enter_context(tc.tile_pool(name="zsb", bufs=2))
            dot_pool = ctx.enter_context(tc.tile_pool(name="dot", bufs=3))
            prod_pool = ctx.enter_context(tc.tile_pool(name="prod", bufs=2))
            small_pool = ctx.enter_context(tc.tile_pool(name="small", bufs=4))

            # ---- persistent accumulators ----
            res_ps0 = res_pool.tile([BSH, 512], F32, tag="res0")
            res_ps1 = res_pool.tile([BSH, 512], F32, tag="res1")
            se_ps = res_pool.tile([BSH, 2], F32, tag="sumexp")

            p_view = p_d[:].rearrange("(t p) e -> p t e", p=128)
            a_view = att_d[:].rearrange("(t p) e -> p t e", p=128)

            p_tiles = {}
            a_tiles = {}
            GMAX = max(GROUPS)

            def load_p_group(g):
                lo = GSTART[g]
                hi = lo + GROUPS[g]
                t_ = p_pool.tile([128, GMAX * ATTH], BF16, tag="pg")
                nc.gpsimd.dma_start(
                    out=t_[:, 0:(hi - lo) * ATTH].rearrange(
                        "p (t e) -> p t e", e=ATTH),
                    in_=p_view[:, lo:hi, :])
                for t in range(lo, hi):
                    p_tiles[t] = t_[:, (t - lo) * ATTH:(t - lo + 1) * ATTH]

            def load_a_group(g):
                lo = GSTART[g]
                hi = lo + GROUPS[g]
                t_ = a_pool.tile([128, GMAX * RNN], BF16, tag="ag")
                # split halves across two DMA queues (sync + scalar) to
                # stay under the per-queue throughput cap
                view = t_[:, 0:(hi - lo) * RNN].rearrange(
                    "p (t e) -> p t e", e=RNN)
                nc.sync.dma_start(
                    out=view[:, :, 0:512], in_=a_view[:, lo:hi, 0:512])
                nc.scalar.dma_start(
                    out=view[:, :, 512:1024], in_=a_view[:, lo:hi, 512:1024])
                for t in range(lo, hi):
                    a_tiles[t] = t_[:, (t - lo) * RNN:(t - lo + 1) * RNN]

            def score_tile(dslice, scol):
                # scores col = sum_a dot * w_alpha, one fused DVE op
                prod_sb = prod_pool.tile([128, ATTH], BF16, tag="prod")
                nc.vector._custom_dve(
                    TENSOR_TENSOR_REDUCE, out=prod_sb[:], in0=dslice,
                    in1=wb_sb[:], s0=0.0, s1=1.0, imm2=0.0,
                    accum_out=scol)

            # ---- main loop over 49 row-tiles, grouped by DMA group ----
            for _rep, g in _rep_groups(repeats):
                lo = GSTART[g]
                hi = lo + GROUPS[g]
                load_p_group(g)
                load_a_group(g)

                scol_g = small_pool.tile([128, GROUPS[g]], F32, tag="scol")

                # boundary tiles: z in PSUM via TensorE (ident + bsel)
                for t0, t1 in g_bnd[g]:
                    n_z = 1024 if t1 is not None else 512
                    z_ps = zp_pool.tile([128, 1024], F32, tag="z")
                    for i, t in enumerate((t0, t1)):
                        if t is None:
                            continue
                        zslice = z_ps[:, i * 512:(i + 1) * 512]
                        nc.tensor.matmul(
                            zslice, lhsT=ident_sb[:],
                            rhs=p_tiles[t], start=True, stop=False)
                        nc.tensor.matmul(
                            zslice,
                            lhsT=bsel_sb[:, t * 128:(t + 1) * 128],
                            rhs=atth_sb[:],
                            start=False, stop=True)
                    dot_sb = dot_pool.tile([128, 1024], BF16, tag="dot")
                    nc.scalar.activation(dot_sb[:, 0:n_z], z_ps[:, 0:n_z],
                                         AF.Tanh)
                    for i, t in enumerate((t0, t1)):
                        if t is None:
                            continue
                        score_tile(dot_sb[:, i * 512:(i + 1) * 512],
                                   scol_g[:, t - lo: t - lo + 1])

                # clean tiles: z in SBUF via DVE add of broadcast att_h
                for t0, t1 in g_clean[g]:
                    n_z = 1024 if t1 is not None else 512
                    z2_sb = zsb_pool.tile([128, 1024], BF16, tag="z2")
                    for i, t in enumerate((t0, t1)):
                        if t is None:
                            continue
                        nc.vector.tensor_tensor(
                            out=z2_sb[:, i * 512:(i + 1) * 512],
                            in0=p_tiles[t], in1=zbc_slice[t], op=ALU.add)
                    dot_sb = dot_pool.tile([128, 1024], BF16, tag="dot")
                    nc.scalar.activation(dot_sb[:, 0:n_z], z2_sb[:, 0:n_z],
                                         AF.Tanh)
                    for i, t in enumerate((t0, t1)):
                        if t is None:
                            continue
                        score_tile(dot_sb[:, i * 512:(i + 1) * 512],
                                   scol_g[:, t - lo: t - lo + 1])

                # e = exp(scores) for the whole group
                ecol_g = small_pool.tile([128, GROUPS[g]], F32, tag="ecol")
                nc.scalar.activation(ecol_g[:], scol_g[:], AF.Exp)

                for t in range(lo, hi):
                    # masked weight columns: lhsT[p, b] = e[p] * mask[p, b]
                    lhsT_t = small_pool.tile([128, BSH], BF16, tag="lhsT")
                    nc.vector.tensor_scalar(
                        out=lhsT_t[:], in0=maskT_sb[:, t * BSH:(t + 1) * BSH],
                        scalar1=ecol_g[:, t - lo: t - lo + 1], scalar2=None,
                        op0=ALU.mult)

                    # att_res += lhsT.T @ A ; sumexp += lhsT.T @ 1
                    nc.tensor.matmul(
                        res_ps0[:], lhsT=lhsT_t[:],
                        rhs=a_tiles[t][:, 0:512],
                        start=(t == 0), stop=(t == NT - 1))
                    nc.tensor.matmul(
                        res_ps1[:], lhsT=lhsT_t[:],
                        rhs=a_tiles[t][:, 512:1024],
                        start=(t == 0), stop=(t == NT - 1))
                    nc.tensor.matmul(
                        se_ps[:], lhsT=lhsT_t[:], rhs=onesr_sb[:],
                        start=(t == 0), stop=(t == NT - 1))

                # ---- finalize: out = att_res / sumexp (per repeat so no
                # repeat is dead code in benchmark builds) ----
                if g == len(GROUPS) - 1:
                    recip_sb = work.tile([BSH, 1], F32)
                    nc.vector.reciprocal(recip_sb[:], se_ps[:, 0:1])
                    out_sb = work.tile([BSH, RNN], F32)
                    nc.scalar.activation(out_sb[:, 0:512], res_ps0[:], AF.Copy,
                                         bias=0.0, scale=recip_sb[:, 0:1])
                    nc.sync.dma_start(out=out_d[:, 0:512],
                                      in_=out_sb[:, 0:512])
                    nc.scalar.activation(out_sb[:, 512:1024], res_ps1[:],
                                         AF.Copy,
                                         bias=0.0, scale=recip_sb[:, 0:1])
                    nc.sync.dma_start(out=out_d[:, 512:1024],
                                      in_=out_sb[:, 512:1024])

    nc.compile()
    return nc


def make_in_maps(h, att_feats, p_att_feats, w_h2att, b_h2att, w_alpha):
    """Host-side prep: shard over batch, cast to bf16, pre-transpose W/h."""
    h = np.asarray(h, dtype=np.float32)
    att_feats = np.asarray(att_feats, dtype=np.float32)
    p_att_feats = np.asarray(p_att_feats, dtype=np.float32)
    w_h2att = np.asarray(w_h2att, dtype=np.float32)
    b_h2att = np.asarray(b_h2att, dtype=np.float32).reshape(1, ATTH)
    w_alpha = np.asarray(w_alpha, dtype=np.float32).reshape(1, ATTH)

    # wT[r, a] chunked over 8 r-chunks of 128: [128, 8*ATTH]
    wT = np.ascontiguousarray(
        w_h2att.T.reshape(8, 128, ATTH).transpose(1, 0, 2).reshape(
            128, 8 * ATTH)).astype(BF)
    bias16 = b_h2att.astype(BF)
    walpha16 = w_alpha.astype(BF)

    in_maps = []
    for c in range(NCORES):
        lo = c * BSH
        hi = lo + BSH
        hT = np.ascontiguousarray(
            h[lo:hi].T.reshape(8, 128, BSH).transpose(1, 0, 2).reshape(
                128, 8 * BSH)).astype(BF)
        in_maps.append({
            "hT": hT,
            "att": np.ascontiguousarray(
                att_feats[lo:hi].reshape(G, RNN)).astype(BF),
            "p_att": np.ascontiguousarray(
                p_att_feats[lo:hi].reshape(G, ATTH)).astype(BF),
            "wT": wT,
            "b_h2att": bias16,
            "w_alpha": walpha16,
        })
    return in_maps


def kernel(h, att_feats, p_att_feats, w_h2att, b_h2att, w_alpha, b_alpha):
    """Full-input entry point. b_alpha is dropped: softmax is shift-invariant."""
    if "nc" not in _cached:
        _cached["nc"] = build_nc()
    nc = _cached["nc"]

    in_maps = make_in_maps(h, att_feats, p_att_feats, w_h2att, b_h2att,
                           w_alpha)
    res = run_bass_kernel_spmd(nc, in_maps, list(range(NCORES)))
    out = np.concatenate([res.results[c]["out"] for c in range(NCORES)],
                         axis=0)
    return out.astype(np.float32)


# revision 34
# speedup vs baseline: 1.1995x; 1.1995x over previous
"""Trainium2 Bass kernel for additive attention (nn_Attention).

Reference computation (per batch b):
    att_h  = h @ W.T + b_h2att                      [B, ATTH]
    dot    = tanh(p_att_feats + att_h[:, None, :])  [B, S, ATTH]
    scores = dot @ w_alpha[0] (+ b_alpha)           [B, S]
    weight = softmax(scores, axis=1)
    out    = weight @ att_feats                     [B, RNN]

Sharding: data-parallel over batch, 32 batches per core x 8 cores.

All inputs are converted to bf16 on the host (DMA is the roofline) and
W/h are host-pre-transposed. Per-core layout: (batch, S) flattened to
G = 32*196 = 6272 rows = exactly 49 tiles of 128 partitions.

TensorE is kept to the irreducible minimum (att_res/sumexp matmuls) so
the kernel stays fast even when the PE clock is thermally throttled:
  - setup builds zbc[t] = att_h[batch(row)] for every tile row via 49
    one-time bsel matmuls (0/1 batch-selector), copied to SBUF bf16.
  - per iteration, z = p + zbc is ONE DVE add per DMA group, and
    dot = tanh(z) is ONE ScalarE call per group.
  - scores col per tile: either a fused custom-DVE TENSOR_TENSOR_REDUCE
    ('v') or a DVE multiply + ScalarE accumulate copy ('a'), pattern
    chosen to balance DVE vs ScalarE occupancy.
  - e = exp(scores); lhsT[p, b] = e[p] * (batch(p)==b) on DVE (bf16)
  - att_res += lhsT.T @ att_tile ; sumexp += lhsT.T @ ones  (TensorE)
Final: out = att_res * (1/sumexp) fused into the PSUM->SBUF copy.
(b_alpha is dropped: softmax is shift-invariant; |scores| <~ 20 so the
unshifted exp stays in fp32 range.)
"""

import numpy as np
import ml_dtypes

import concourse.bass as bass
import concourse.tile as tile
from concourse import bacc, mybir
from concourse.bass_utils import run_bass_kernel_spmd
from concourse.dve_ops import TENSOR_TENSOR_REDUCE

F32 = mybir.dt.float32
BF16 = mybir.dt.bfloat16
FP8 = mybir.dt.float8e4
AF = mybir.ActivationFunctionType
ALU = mybir.AluOpType
BF = ml_dtypes.bfloat16
F8NP = mybir.dt.np(FP8)

B, S, RNN, ATTH = 256, 196, 1024, 512
NCORES = 8
BSH = B // NCORES            # 32 batches per core
G = BSH * S                  # 6272 rows per core
NT = G // 128                # 49 tiles
assert NT * 128 == G

_cached = {}


def _batch_of_row(g):
    return g // S


def build_nc(repeats=1, att_q2="scalar", p_fp8=False, score_pat="vav",
             ablate=None, att_split="col", groups7=False):
    # att_q2: second DMA queue for the att halves; None = single sync queue
    # p_fp8: DMA p_att as fp8e4m3 (halves its bytes; scores lose ~1 digit)
    # score_pat: per-tile scores-reduce engine, indexed by t % len(pat):
    #   'v' = fused custom-DVE op, 'a' = DVE multiply + ScalarE accum
    # ablate="dma": loads + finalize only (timing probe, wrong results)
    # att_split: "col" = halve by feature columns (1KB lines),
    #            "part" = halve by partition range (full 2KB lines)
    # groups7: 7 DMA groups of 7 tiles instead of [4,5,6x6,4]
    PDT = FP8 if p_fp8 else BF16
    GROUPS = [7] * 7 if groups7 else [4, 5, 6, 6, 6, 6, 6, 6, 4]
    assert sum(GROUPS) == NT
    GSTART = [sum(GROUPS[:i]) for i in range(len(GROUPS))]

    def _rep_groups(repeats):
        for r in range(repeats):
            for g in range(len(GROUPS)):
                yield r, g
    nc = bacc.Bacc("TRN2", target_bir_lowering=False, debug=False,
                   enable_asserts=True, num_devices=NCORES)

    hT_d = nc.dram_tensor("hT", [128, 8 * BSH], BF16, kind="ExternalInput")
    att_d = nc.dram_tensor("att", [G, RNN], BF16, kind="ExternalInput")
    p_d = nc.dram_tensor("p_att", [G, ATTH], PDT, kind="ExternalInput")
    wT_d = nc.dram_tensor("wT", [128, 8 * ATTH], BF16, kind="ExternalInput")
    bias_d = nc.dram_tensor("b_h2att", [1, ATTH], BF16, kind="ExternalInput")
    walpha_d = nc.dram_tensor("w_alpha", [1, ATTH], BF16,
                              kind="ExternalInput")
    out_d = nc.dram_tensor("out", [BSH, RNN], F32, kind="ExternalOutput")

    # --- host-side constants, embedded in the NEFF ---
    ones_np = np.ones((128, 128), dtype=np.float32).astype(BF)
    # maskT[p, t, b] = 1 if batch(128t + p) == b
    maskT_np = np.zeros((128, NT, BSH), dtype=np.float32)
    for t in range(NT):
        for p in range(128):
            bb = _batch_of_row(128 * t + p)
            maskT_np[p, t, bb] = 1.0
    # bsel[b, t, p]: one-hot selector, bsel.T @ att_h broadcasts per-row att_h
    bsel_np = np.ascontiguousarray(maskT_np.transpose(2, 1, 0)).astype(BF)
    maskT_np = maskT_np.astype(BF)

    ones_c = nc.inline_tensor(ones_np, "c_ones")
    bsel_c = nc.inline_tensor(bsel_np.reshape(BSH, NT * 128), "c_bsel")
    maskT_c = nc.inline_tensor(maskT_np.reshape(128, NT * BSH), "c_maskT")

    with tile.TileContext(nc) as tc:
        import contextlib
        ctx = contextlib.ExitStack()
        with ctx:
            consts = ctx.enter_context(tc.tile_pool(name="consts", bufs=1))
            work = ctx.enter_context(tc.tile_pool(name="work", bufs=1))
            zbc_pool = ctx.enter_context(tc.tile_pool(name="zbc", bufs=1))
            p_pool = ctx.enter_context(tc.tile_pool(name="p_pool", bufs=2))
            a_pool = ctx.enter_context(tc.tile_pool(name="a_pool", bufs=4))
            setup_sb_cm = tc.tile_pool(name="setup_sb", bufs=1)
            setup_sb = setup_sb_cm.__enter__()
            res_pool = ctx.enter_context(
                tc.tile_pool(name="respsum", bufs=1, space="PSUM"))
            setup_ps_cm = tc.tile_pool(name="setupps", bufs=2, space="PSUM")
            setup_ps = setup_ps_cm.__enter__()

            # ---- load constants and small inputs ----
            hT_sb = setup_sb.tile([128, 8 * BSH], BF16)
            nc.gpsimd.dma_start(out=hT_sb[:], in_=hT_d[:])
            wT_sb = setup_sb.tile([128, 8 * ATTH], BF16)
            nc.gpsimd.dma_start(out=wT_sb[:], in_=wT_d[:])
            ones_sb = consts.tile([128, 128], BF16)
            nc.scalar.dma_start(out=ones_sb[:], in_=ones_c[:])
            onesr_sb = consts.tile([128, 2], BF16)
            nc.scalar.dma_start(out=onesr_sb[:], in_=ones_c[:, 0:2])
            bias_sb = setup_sb.tile([1, ATTH], BF16)
            nc.scalar.dma_start(out=bias_sb[:], in_=bias_d[:])
            walpha_sb = setup_sb.tile([1, ATTH], BF16)
            nc.scalar.dma_start(out=walpha_sb[:], in_=walpha_d[:])
            bsel_sb = setup_sb.tile([BSH, NT * 128], BF16)
            nc.scalar.dma_start(out=bsel_sb[:], in_=bsel_c[:])
            maskT_sb = consts.tile([128, NT * BSH], BF16)
            nc.scalar.dma_start(out=maskT_sb[:], in_=maskT_c[:])

            # ---- att_h = h @ W.T + bias via pre-transposed hT/wT ----
            ah_full = setup_ps.tile([128, 1024], F32, tag="zb")
            ah_ps = ah_full[0:BSH, 0:ATTH]
            for rc in range(8):
                nc.tensor.matmul(
                    ah_ps[:],
                    lhsT=hT_sb[:, rc * BSH:(rc + 1) * BSH],
                    rhs=wT_sb[:, rc * ATTH:(rc + 1) * ATTH],
                    start=(rc == 0), stop=False)
            nc.tensor.matmul(
                ah_ps[:], lhsT=ones_sb[0:1, 0:BSH], rhs=bias_sb[0:1, :],
                start=False, stop=True)
            atth_sb = work.tile([BSH, ATTH], BF16)
            nc.vector.tensor_copy(atth_sb[:], ah_ps[:])

            # ---- broadcast w_alpha to all 128 partitions (bf16) ----
            wb_full = setup_ps.tile([128, 1024], F32, tag="zb")
            wb_ps = wb_full[:, 0:ATTH]
            nc.tensor.matmul(wb_ps[:], lhsT=ones_sb[0:1, 0:128],
                             rhs=walpha_sb[0:1, :], start=True, stop=True)
            wb_sb = work.tile([128, ATTH], BF16)
            nc.vector.tensor_copy(wb_sb[:], wb_ps[:])

            # ---- zbc[t][p, :] = att_h[batch(128t+p)], for every tile,
            # built once by TensorE bsel matmuls + copied to SBUF bf16.
            # Stored per DMA group so the per-iteration z-add is one DVE
            # tensor_tensor per group. ----
            zbc_g = {}
            zbc_slice = {}
            for g in range(len(GROUPS)):
                lo, hi = GSTART[g], GSTART[g] + GROUPS[g]
                zt = zbc_pool.tile([128, GROUPS[g] * ATTH], BF16,
                                   tag=f"zbcg{g}", name=f"zbcg{g}")
                zbc_g[g] = zt
                for t in range(lo, hi):
                    zbc_slice[t] = zt[:, (t - lo) * ATTH:(t - lo + 1) * ATTH]
                for i in range(0, GROUPS[g], 2):
                    t0 = lo + i
                    t1 = t0 + 1 if i + 1 < GROUPS[g] else None
                    n_z = 1024 if t1 is not None else 512
                    zb_ps = setup_ps.tile([128, 1024], F32, tag="zb")
                    for j, t in enumerate((t0, t1)):
                        if t is None:
                            continue
                        nc.tensor.matmul(
                            zb_ps[:, j * 512:(j + 1) * 512],
                            lhsT=bsel_sb[:, t * 128:(t + 1) * 128],
                            rhs=atth_sb[:], start=True, stop=True)
                    dst = zt[:, i * ATTH: i * ATTH + n_z]
                    if (i // 2) % 2 == 0:
                        nc.vector.tensor_copy(dst, zb_ps[:, 0:n_z])
                    else:
                        nc.scalar.activation(dst, zb_ps[:, 0:n_z], AF.Copy)
            setup_sb_cm.__exit__(None, None, None)
            setup_ps_cm.__exit__(None, None, None)

            zsb_pool = ctx.enter_context(tc.tile_pool(name="zsb", bufs=2))
            dot_pool = ctx.enter_context(tc.tile_pool(name="dot", bufs=2))
            prod_pool = ctx.enter_context(tc.tile_pool(name="prod", bufs=2))
            small_pool = ctx.enter_context(tc.tile_pool(name="small", bufs=4))

            # ---- persistent accumulators ----
            res_ps0 = res_pool.tile([BSH, 512], F32, tag="res0")
            res_ps1 = res_pool.tile([BSH, 512], F32, tag="res1")
            se_ps = res_pool.tile([BSH, 2], F32, tag="sumexp")

            p_view = p_d[:].rearrange("(t p) e -> p t e", p=128)
            a_view = att_d[:].rearrange("(t p) e -> p t e", p=128)

            p_tiles = {}
            p_gtile = {}
            a_tiles = {}
            GMAX = max(GROUPS)

            def load_p_group(g):
                lo = GSTART[g]
                hi = lo + GROUPS[g]
                # gpsimd (software DGE) DMAs can cast: with p_fp8 the HBM
                # side is fp8e4m3 but SBUF receives bf16, so the DVE z-add
                # keeps its 2x 16-bit mode.
                t_ = p_pool.tile([128, GMAX * ATTH], BF16, tag="pg")
                nc.gpsimd.dma_start(
                    out=t_[:, 0:(hi - lo) * ATTH].rearrange(
                        "p (t e) -> p t e", e=ATTH),
                    in_=p_view[:, lo:hi, :])
                p_gtile[g] = t_
                for t in range(lo, hi):
                    p_tiles[t] = t_[:, (t - lo) * ATTH:(t - lo + 1) * ATTH]

            def load_a_group(g):
                lo = GSTART[g]
                hi = lo + GROUPS[g]
                t_ = a_pool.tile([128, GMAX * RNN], BF16, tag="ag")
                # optionally split halves across two DMA queues to stay
                # under the per-queue throughput cap
                view = t_[:, 0:(hi - lo) * RNN].rearrange(
                    "p (t e) -> p t e", e=RNN)
                if att_q2 is None:
                    nc.sync.dma_start(out=view[:], in_=a_view[:, lo:hi, :])
                elif att_split == "part":
                    q2 = getattr(nc, att_q2)
                    nc.sync.dma_start(
                        out=view[0:64], in_=a_view[0:64, lo:hi, :])
                    q2.dma_start(
                        out=view[64:128], in_=a_view[64:128, lo:hi, :])
                else:
                    q2 = getattr(nc, att_q2)
                    nc.sync.dma_start(
                        out=view[:, :, 0:512], in_=a_view[:, lo:hi, 0:512])
                    q2.dma_start(
                        out=view[:, :, 512:1024],
                        in_=a_view[:, lo:hi, 512:1024])
                for t in range(lo, hi):
                    a_tiles[t] = t_[:, (t - lo) * RNN:(t - lo + 1) * RNN]

            # ---- main loop over 49 row-tiles, grouped by DMA group ----
            for _rep, g in _rep_groups(repeats):
                lo = GSTART[g]
                hi = lo + GROUPS[g]
                ng = hi - lo
                load_p_group(g)
                load_a_group(g)

                if ablate == "dma":
                    if g == len(GROUPS) - 1:
                        out_sb = work.tile([BSH, RNN], F32)
                        # touch the loaded tiles so DMA isn't dead-code
                        nc.vector.tensor_copy(
                            out_sb[:], a_tiles[lo][0:BSH, :])
                        nc.sync.dma_start(out=out_d[:], in_=out_sb[:])
                    continue

                # z = p + zbc: one DVE add per group; dot = tanh(z): one
                # ScalarE call per group
                z_sb = zsb_pool.tile([128, GMAX * ATTH], BF16, tag="z")
                nc.vector.tensor_tensor(
                    out=z_sb[:, 0:ng * ATTH],
                    in0=p_gtile[g][:, 0:ng * ATTH],
                    in1=zbc_g[g][:], op=ALU.add)
                dot_sb = dot_pool.tile([128, GMAX * ATTH], BF16, tag="dot")
                nc.scalar.activation(dot_sb[:, 0:ng * ATTH],
                                     z_sb[:, 0:ng * ATTH], AF.Tanh)

                scol_g = small_pool.tile([128, GROUPS[g]], F32, tag="scol")
                for t in range(lo, hi):
                    dslice = dot_sb[:, (t - lo) * ATTH:(t - lo + 1) * ATTH]
                    scol = scol_g[:, t - lo: t - lo + 1]
                    prod_sb = prod_pool.tile([128, ATTH], BF16, tag="prod")
                    if score_pat[t % len(score_pat)] == "v":
                        # fused multiply+reduce on DVE
                        nc.vector._custom_dve(
                            TENSOR_TENSOR_REDUCE, out=prod_sb[:],
                            in0=dslice, in1=wb_sb[:], s0=0.0, s1=1.0,
                            imm2=0.0, accum_out=scol)
                    else:
                        # DVE multiply + ScalarE accumulate copy
                        nc.vector.tensor_tensor(
                            out=prod_sb[:], in0=dslice, in1=wb_sb[:],
                            op=ALU.mult)
                        nc.scalar.activation(
                            dslice, prod_sb[:], AF.Copy, bias=0.0,
                            scale=1.0, accum_out=scol)

                # e = exp(scores) for the whole group
                ecol_g = small_pool.tile([128, GROUPS[g]], F32, tag="ecol")
                nc.scalar.activation(ecol_g[:], scol_g[:], AF.Exp)

                for t in range(lo, hi):
                    # masked weight columns: lhsT[p, b] = e[p] * mask[p, b]
                    lhsT_t = small_pool.tile([128, BSH], BF16, tag="lhsT")
                    nc.vector.tensor_scalar(
                        out=lhsT_t[:], in0=maskT_sb[:, t * BSH:(t + 1) * BSH],
                        scalar1=ecol_g[:, t - lo: t - lo + 1], scalar2=None,
                        op0=ALU.mult)

                    # att_res += lhsT.T @ A ; sumexp += lhsT.T @ 1
                    nc.tensor.matmul(
                        res_ps0[:], lhsT=lhsT_t[:],
                        rhs=a_tiles[t][:, 0:512],
                        start=(t == 0), stop=(t == NT - 1))
                    nc.tensor.matmul(
                        res_ps1[:], lhsT=lhsT_t[:],
                        rhs=a_tiles[t][:, 512:1024],
                        start=(t == 0), stop=(t == NT - 1))
                    nc.tensor.matmul(
                        se_ps[:], lhsT=lhsT_t[:], rhs=onesr_sb[:],
                        start=(t == 0), stop=(t == NT - 1))

                # ---- finalize: out = att_res / sumexp (per repeat so no
                # repeat is dead code in benchmark builds) ----
                if g == len(GROUPS) - 1:
                    recip_sb = work.tile([BSH, 1], F32)
                    nc.vector.reciprocal(recip_sb[:], se_ps[:, 0:1])
                    out_sb = work.tile([BSH, RNN], F32)
                    nc.scalar.activation(out_sb[:, 0:512], res_ps0[:], AF.Copy,
                                         bias=0.0, scale=recip_sb[:, 0:1])
                    nc.sync.dma_start(out=out_d[:, 0:512],
                                      in_=out_sb[:, 0:512])
                    nc.scalar.activation(out_sb[:, 512:1024], res_ps1[:],
                                         AF.Copy,
                                         bias=0.0, scale=recip_sb[:, 0:1])
                    nc.sync.dma_start(out=out_d[:, 512:1024],
                                      in_=out_sb[:, 512:1024])

    nc.compile()
    return nc


def make_in_maps(h, att_feats, p_att_feats, w_h2att, b_h2att, w_alpha,
                 p_fp8=False):
    """Host-side prep: shard over batch, cast to bf16, pre-transpose W/h."""
    PNP = F8NP if p_fp8 else BF
    h = np.asarray(h, dtype=np.float32)
    att_feats = np.asarray(att_feats, dtype=np.float32)
    p_att_feats = np.asarray(p_att_feats, dtype=np.float32)
    w_h2att = np.asarray(w_h2att, dtype=np.float32)
    b_h2att = np.asarray(b_h2att, dtype=np.float32).reshape(1, ATTH)
    w_alpha = np.asarray(w_alpha, dtype=np.float32).reshape(1, ATTH)

    # wT[r, a] chunked over 8 r-chunks of 128: [128, 8*ATTH]
    wT = np.ascontiguousarray(
        w_h2att.T.reshape(8, 128, ATTH).transpose(1, 0, 2).reshape(
            128, 8 * ATTH)).astype(BF)
    bias16 = b_h2att.astype(BF)
    walpha16 = w_alpha.astype(BF)

    in_maps = []
    for c in range(NCORES):
        lo = c * BSH
        hi = lo + BSH
        hT = np.ascontiguousarray(
            h[lo:hi].T.reshape(8, 128, BSH).transpose(1, 0, 2).reshape(
                128, 8 * BSH)).astype(BF)
        in_maps.append({
            "hT": hT,
            "att": np.ascontiguousarray(
                att_feats[lo:hi].reshape(G, RNN)).astype(BF),
            "p_att": np.ascontiguousarray(
                p_att_feats[lo:hi].reshape(G, ATTH)).astype(PNP),
            "wT": wT,
            "b_h2att": bias16,
            "w_alpha": walpha16,
        })
    return in_maps


# production build configuration (kernel() and test.py timing use this)
BUILD_KW = dict(att_q2="scalar", p_fp8=False, score_pat="vav")


def kernel(h, att_feats, p_att_feats, w_h2att, b_h2att, w_alpha, b_alpha):
    """Full-input entry point. b_alpha is dropped: softmax is shift-invariant."""
    if "nc" not in _cached:
        _cached["nc"] = build_nc(**BUILD_KW)
    nc = _cached["nc"]

    in_maps = make_in_maps(h, att_feats, p_att_feats, w_h2att, b_h2att,
                           w_alpha, p_fp8=BUILD_KW["p_fp8"])
    res = run_bass_kernel_spmd(nc, in_maps, list(range(NCORES)))
    out = np.concatenate([res.results[c]["out"] for c in range(NCORES)],
                         axis=0)
    return out.astype(np.float32)


# revision 40
# speedup vs baseline: 1.2943x; 1.0790x over previous
"""Trainium2 Bass kernel for additive attention (nn_Attention).

Reference computation (per batch b):
    att_h  = h @ W.T + b_h2att                      [B, ATTH]
    dot    = tanh(p_att_feats + att_h[:, None, :])  [B, S, ATTH]
    scores = dot @ w_alpha[0] (+ b_alpha)           [B, S]
    weight = softmax(scores, axis=1)
    out    = weight @ att_feats                     [B, RNN]

Sharding: data-parallel over batch, 32 batches per core x 8 cores.

All inputs are converted to bf16 on the host (DMA is the roofline) and
W/h are host-pre-transposed. Per-core layout: (batch, S) flattened to
G = 32*196 = 6272 rows = exactly 49 tiles of 128 partitions.

TensorE is kept to the irreducible minimum (att_res/sumexp matmuls) so
the kernel stays fast even when the PE clock is thermally throttled:
  - setup builds zbc[t] = att_h[batch(row)] for every tile row via 49
    one-time bsel matmuls (0/1 batch-selector), copied to SBUF bf16.
  - per iteration, z = p + zbc is ONE DVE add per DMA group, and
    dot = tanh(z) is ONE ScalarE call per group.
  - scores col per tile: either a fused custom-DVE TENSOR_TENSOR_REDUCE
    ('v') or a DVE multiply + ScalarE accumulate copy ('a'), pattern
    chosen to balance DVE vs ScalarE occupancy.
  - e = exp(scores); lhsT[p, b] = e[p] * (batch(p)==b) on DVE (bf16)
  - att_res += lhsT.T @ att_tile ; sumexp += lhsT.T @ ones  (TensorE)
Final: out = att_res * (1/sumexp) fused into the PSUM->SBUF copy.
(b_alpha is dropped: softmax is shift-invariant; |scores| <~ 20 so the
unshifted exp stays in fp32 range.)
"""

import numpy as np
import ml_dtypes

import concourse.bass as bass
import concourse.tile as tile
from concourse import bacc, mybir
from concourse.bass_utils import run_bass_kernel_spmd
from concourse.dve_ops import TENSOR_TENSOR_REDUCE

F32 = mybir.dt.float32
BF16 = mybir.dt.bfloat16
FP8 = mybir.dt.float8e4
AF = mybir.ActivationFunctionType
ALU = mybir.AluOpType
BF = ml_dtypes.bfloat16
F8NP = mybir.dt.np(FP8)

B, S, RNN, ATTH = 256, 196, 1024, 512
NCORES = 8
BSH = B // NCORES            # 32 batches per core
G = BSH * S                  # 6272 rows per core
NT = G // 128                # 49 tiles
assert NT * 128 == G

_cached = {}


def _batch_of_row(g):
    return g // S


def build_nc(repeats=1, att_q2="scalar", p_fp8=False, score_pat="vav",
             ablate=None, att_split="col", groups7=False,
             lhsT_pool=False, prefetch=False, deep_bufs=False):
    # lhsT_pool: build lhsT masked-weight columns on GPSIMD instead of DVE
    # prefetch: issue group g+1's DMA loads before group g's compute so
    #   queue-order never blocks the next transfer behind compute deps
    # att_q2: second DMA queue for the att halves; None = single sync queue
    # p_fp8: DMA p_att as fp8e4m3 (halves its bytes; scores lose ~1 digit)
    # score_pat: per-tile scores-reduce engine, indexed by t % len(pat):
    #   'v' = fused custom-DVE op, 'a' = DVE multiply + ScalarE accum
    # ablate="dma": loads + finalize only (timing probe, wrong results)
    # att_split: "col" = halve by feature columns (1KB lines),
    #            "part" = halve by partition range (full 2KB lines)
    # groups7: 7 DMA groups of 7 tiles instead of [4,5,6x6,4]
    PDT = FP8 if p_fp8 else BF16
    GROUPS = [7] * 7 if groups7 else [4, 5, 6, 6, 6, 6, 6, 6, 4]
    assert sum(GROUPS) == NT
    GSTART = [sum(GROUPS[:i]) for i in range(len(GROUPS))]

    def _rep_groups(repeats):
        for r in range(repeats):
            for g in range(len(GROUPS)):
                yield r, g
    nc = bacc.Bacc("TRN2", target_bir_lowering=False, debug=False,
                   enable_asserts=True, num_devices=NCORES)

    hT_d = nc.dram_tensor("hT", [128, 8 * BSH], BF16, kind="ExternalInput")
    att_d = nc.dram_tensor("att", [G, RNN], BF16, kind="ExternalInput")
    p_d = nc.dram_tensor("p_att", [G, ATTH], PDT, kind="ExternalInput")
    wT_d = nc.dram_tensor("wT", [128, 8 * ATTH], BF16, kind="ExternalInput")
    bias_d = nc.dram_tensor("b_h2att", [1, ATTH], BF16, kind="ExternalInput")
    walpha_d = nc.dram_tensor("w_alpha", [1, ATTH], BF16,
                              kind="ExternalInput")
    out_d = nc.dram_tensor("out", [BSH, RNN], F32, kind="ExternalOutput")

    # --- host-side constants, embedded in the NEFF ---
    ones_np = np.ones((128, 128), dtype=np.float32).astype(BF)
    # maskT[p, t, b] = 1 if batch(128t + p) == b
    maskT_np = np.zeros((128, NT, BSH), dtype=np.float32)
    for t in range(NT):
        for p in range(128):
            bb = _batch_of_row(128 * t + p)
            maskT_np[p, t, bb] = 1.0
    # bsel[b, t, p]: one-hot selector, bsel.T @ att_h broadcasts per-row att_h
    bsel_np = np.ascontiguousarray(maskT_np.transpose(2, 1, 0)).astype(BF)
    maskT_np = maskT_np.astype(BF)

    ones_c = nc.inline_tensor(ones_np, "c_ones")
    bsel_c = nc.inline_tensor(bsel_np.reshape(BSH, NT * 128), "c_bsel")
    maskT_c = nc.inline_tensor(maskT_np.reshape(128, NT * BSH), "c_maskT")

    with tile.TileContext(nc) as tc:
        import contextlib
        ctx = contextlib.ExitStack()
        with ctx:
            consts = ctx.enter_context(tc.tile_pool(name="consts", bufs=1))
            work = ctx.enter_context(tc.tile_pool(name="work", bufs=1))
            zbc_pool = ctx.enter_context(tc.tile_pool(name="zbc", bufs=1))
            p_pool = ctx.enter_context(
                tc.tile_pool(name="p_pool", bufs=3 if deep_bufs else 2))
            a_pool = ctx.enter_context(
                tc.tile_pool(name="a_pool", bufs=6 if deep_bufs else 4))
            setup_sb_cm = tc.tile_pool(name="setup_sb", bufs=1)
            setup_sb = setup_sb_cm.__enter__()
            res_pool = ctx.enter_context(
                tc.tile_pool(name="respsum", bufs=1, space="PSUM"))
            setup_ps_cm = tc.tile_pool(name="setupps", bufs=2, space="PSUM")
            setup_ps = setup_ps_cm.__enter__()

            # ---- load constants and small inputs ----
            hT_sb = setup_sb.tile([128, 8 * BSH], BF16)
            nc.gpsimd.dma_start(out=hT_sb[:], in_=hT_d[:])
            wT_sb = setup_sb.tile([128, 8 * ATTH], BF16)
            nc.gpsimd.dma_start(out=wT_sb[:], in_=wT_d[:])
            ones_sb = consts.tile([128, 128], BF16)
            nc.scalar.dma_start(out=ones_sb[:], in_=ones_c[:])
            onesr_sb = consts.tile([128, 2], BF16)
            nc.scalar.dma_start(out=onesr_sb[:], in_=ones_c[:, 0:2])
            bias_sb = setup_sb.tile([1, ATTH], BF16)
            nc.scalar.dma_start(out=bias_sb[:], in_=bias_d[:])
            walpha_sb = setup_sb.tile([1, ATTH], BF16)
            nc.scalar.dma_start(out=walpha_sb[:], in_=walpha_d[:])
            bsel_sb = setup_sb.tile([BSH, NT * 128], BF16)
            nc.scalar.dma_start(out=bsel_sb[:], in_=bsel_c[:])
            maskT_sb = consts.tile([128, NT * BSH], BF16)
            nc.scalar.dma_start(out=maskT_sb[:], in_=maskT_c[:])

            # ---- att_h = h @ W.T + bias via pre-transposed hT/wT ----
            ah_full = setup_ps.tile([128, 1024], F32, tag="zb")
            ah_ps = ah_full[0:BSH, 0:ATTH]
            for rc in range(8):
                nc.tensor.matmul(
                    ah_ps[:],
                    lhsT=hT_sb[:, rc * BSH:(rc + 1) * BSH],
                    rhs=wT_sb[:, rc * ATTH:(rc + 1) * ATTH],
                    start=(rc == 0), stop=False)
            nc.tensor.matmul(
                ah_ps[:], lhsT=ones_sb[0:1, 0:BSH], rhs=bias_sb[0:1, :],
                start=False, stop=True)
            atth_sb = work.tile([BSH, ATTH], BF16)
            nc.vector.tensor_copy(atth_sb[:], ah_ps[:])

            # ---- broadcast w_alpha to all 128 partitions (bf16) ----
            wb_full = setup_ps.tile([128, 1024], F32, tag="zb")
            wb_ps = wb_full[:, 0:ATTH]
            nc.tensor.matmul(wb_ps[:], lhsT=ones_sb[0:1, 0:128],
                             rhs=walpha_sb[0:1, :], start=True, stop=True)
            wb_sb = work.tile([128, ATTH], BF16)
            nc.vector.tensor_copy(wb_sb[:], wb_ps[:])

            # ---- zbc[t][p, :] = att_h[batch(128t+p)], for every tile,
            # built once by TensorE bsel matmuls + copied to SBUF bf16.
            # Stored per DMA group so the per-iteration z-add is one DVE
            # tensor_tensor per group. ----
            zbc_g = {}
            zbc_slice = {}
            for g in range(len(GROUPS)):
                lo, hi = GSTART[g], GSTART[g] + GROUPS[g]
                zt = zbc_pool.tile([128, GROUPS[g] * ATTH], BF16,
                                   tag=f"zbcg{g}", name=f"zbcg{g}")
                zbc_g[g] = zt
                for t in range(lo, hi):
                    zbc_slice[t] = zt[:, (t - lo) * ATTH:(t - lo + 1) * ATTH]
                for i in range(0, GROUPS[g], 2):
                    t0 = lo + i
                    t1 = t0 + 1 if i + 1 < GROUPS[g] else None
                    n_z = 1024 if t1 is not None else 512
                    zb_ps = setup_ps.tile([128, 1024], F32, tag="zb")
                    for j, t in enumerate((t0, t1)):
                        if t is None:
                            continue
                        nc.tensor.matmul(
                            zb_ps[:, j * 512:(j + 1) * 512],
                            lhsT=bsel_sb[:, t * 128:(t + 1) * 128],
                            rhs=atth_sb[:], start=True, stop=True)
                    dst = zt[:, i * ATTH: i * ATTH + n_z]
                    if (i // 2) % 2 == 0:
                        nc.vector.tensor_copy(dst, zb_ps[:, 0:n_z])
                    else:
                        nc.scalar.activation(dst, zb_ps[:, 0:n_z], AF.Copy)
            setup_sb_cm.__exit__(None, None, None)
            setup_ps_cm.__exit__(None, None, None)

            zsb_pool = ctx.enter_context(tc.tile_pool(name="zsb", bufs=2))
            dot_pool = ctx.enter_context(tc.tile_pool(name="dot", bufs=2))
            prod_pool = ctx.enter_context(tc.tile_pool(name="prod", bufs=2))
            small_pool = ctx.enter_context(tc.tile_pool(name="small", bufs=4))

            # ---- persistent accumulators ----
            res_ps0 = res_pool.tile([BSH, 512], F32, tag="res0")
            res_ps1 = res_pool.tile([BSH, 512], F32, tag="res1")
            se_ps = res_pool.tile([BSH, 2], F32, tag="sumexp")

            p_view = p_d[:].rearrange("(t p) e -> p t e", p=128)
            a_view = att_d[:].rearrange("(t p) e -> p t e", p=128)

            p_tiles = {}
            p_gtile = {}
            a_tiles = {}
            GMAX = max(GROUPS)

            def load_p_group(g):
                lo = GSTART[g]
                hi = lo + GROUPS[g]
                # gpsimd (software DGE) DMAs can cast: with p_fp8 the HBM
                # side is fp8e4m3 but SBUF receives bf16, so the DVE z-add
                # keeps its 2x 16-bit mode.
                t_ = p_pool.tile([128, GMAX * ATTH], BF16, tag="pg")
                nc.gpsimd.dma_start(
                    out=t_[:, 0:(hi - lo) * ATTH].rearrange(
                        "p (t e) -> p t e", e=ATTH),
                    in_=p_view[:, lo:hi, :])
                p_gtile[g] = t_
                for t in range(lo, hi):
                    p_tiles[t] = t_[:, (t - lo) * ATTH:(t - lo + 1) * ATTH]

            def load_a_group(g):
                lo = GSTART[g]
                hi = lo + GROUPS[g]
                t_ = a_pool.tile([128, GMAX * RNN], BF16, tag="ag")
                # optionally split halves across two DMA queues to stay
                # under the per-queue throughput cap
                view = t_[:, 0:(hi - lo) * RNN].rearrange(
                    "p (t e) -> p t e", e=RNN)
                if att_q2 is None:
                    nc.sync.dma_start(out=view[:], in_=a_view[:, lo:hi, :])
                elif att_split == "part":
                    q2 = getattr(nc, att_q2)
                    nc.sync.dma_start(
                        out=view[0:64], in_=a_view[0:64, lo:hi, :])
                    q2.dma_start(
                        out=view[64:128], in_=a_view[64:128, lo:hi, :])
                else:
                    q2 = getattr(nc, att_q2)
                    nc.sync.dma_start(
                        out=view[:, :, 0:512], in_=a_view[:, lo:hi, 0:512])
                    q2.dma_start(
                        out=view[:, :, 512:1024],
                        in_=a_view[:, lo:hi, 512:1024])
                for t in range(lo, hi):
                    a_tiles[t] = t_[:, (t - lo) * RNN:(t - lo + 1) * RNN]

            # ---- main loop over 49 row-tiles, grouped by DMA group ----
            seq = list(_rep_groups(repeats))
            if prefetch:
                load_p_group(seq[0][1])
                load_a_group(seq[0][1])
            for idx, (_rep, g) in enumerate(seq):
                lo = GSTART[g]
                hi = lo + GROUPS[g]
                ng = hi - lo
                if prefetch:
                    if idx + 1 < len(seq):
                        load_p_group(seq[idx + 1][1])
                        load_a_group(seq[idx + 1][1])
                else:
                    load_p_group(g)
                    load_a_group(g)

                if ablate == "dma":
                    if g == len(GROUPS) - 1:
                        out_sb = work.tile([BSH, RNN], F32)
                        # touch the loaded tiles so DMA isn't dead-code
                        nc.vector.tensor_copy(
                            out_sb[:], a_tiles[lo][0:BSH, :])
                        nc.sync.dma_start(out=out_d[:], in_=out_sb[:])
                    continue

                # z = p + zbc: one DVE add per group; dot = tanh(z): one
                # ScalarE call per group
                z_sb = zsb_pool.tile([128, GMAX * ATTH], BF16, tag="z")
                nc.vector.tensor_tensor(
                    out=z_sb[:, 0:ng * ATTH],
                    in0=p_gtile[g][:, 0:ng * ATTH],
                    in1=zbc_g[g][:], op=ALU.add)
                dot_sb = dot_pool.tile([128, GMAX * ATTH], BF16, tag="dot")
                nc.scalar.activation(dot_sb[:, 0:ng * ATTH],
                                     z_sb[:, 0:ng * ATTH], AF.Tanh)

                scol_g = small_pool.tile([128, GROUPS[g]], F32, tag="scol")
                for t in range(lo, hi):
                    dslice = dot_sb[:, (t - lo) * ATTH:(t - lo + 1) * ATTH]
                    scol = scol_g[:, t - lo: t - lo + 1]
                    prod_sb = prod_pool.tile([128, ATTH], BF16, tag="prod")
                    if score_pat[t % len(score_pat)] == "v":
                        # fused multiply+reduce on DVE
                        nc.vector._custom_dve(
                            TENSOR_TENSOR_REDUCE, out=prod_sb[:],
                            in0=dslice, in1=wb_sb[:], s0=0.0, s1=1.0,
                            imm2=0.0, accum_out=scol)
                    else:
                        # DVE multiply + ScalarE accumulate copy
                        nc.vector.tensor_tensor(
                            out=prod_sb[:], in0=dslice, in1=wb_sb[:],
                            op=ALU.mult)
                        nc.scalar.activation(
                            dslice, prod_sb[:], AF.Copy, bias=0.0,
                            scale=1.0, accum_out=scol)

                # e = exp(scores) for the whole group
                ecol_g = small_pool.tile([128, GROUPS[g]], F32, tag="ecol")
                nc.scalar.activation(ecol_g[:], scol_g[:], AF.Exp)

                for t in range(lo, hi):
                    # masked weight columns: lhsT[p, b] = e[p] * mask[p, b]
                    lhsT_t = small_pool.tile([128, BSH], BF16, tag="lhsT")
                    ts_eng = nc.gpsimd if lhsT_pool else nc.vector
                    ts_eng.tensor_scalar(
                        out=lhsT_t[:], in0=maskT_sb[:, t * BSH:(t + 1) * BSH],
                        scalar1=ecol_g[:, t - lo: t - lo + 1], scalar2=None,
                        op0=ALU.mult)

                    # att_res += lhsT.T @ A ; sumexp += lhsT.T @ 1
                    nc.tensor.matmul(
                        res_ps0[:], lhsT=lhsT_t[:],
                        rhs=a_tiles[t][:, 0:512],
                        start=(t == 0), stop=(t == NT - 1))
                    nc.tensor.matmul(
                        res_ps1[:], lhsT=lhsT_t[:],
                        rhs=a_tiles[t][:, 512:1024],
                        start=(t == 0), stop=(t == NT - 1))
                    nc.tensor.matmul(
                        se_ps[:], lhsT=lhsT_t[:], rhs=onesr_sb[:],
                        start=(t == 0), stop=(t == NT - 1))

                # ---- finalize: out = att_res / sumexp (per repeat so no
                # repeat is dead code in benchmark builds) ----
                if g == len(GROUPS) - 1:
                    recip_sb = work.tile([BSH, 1], F32)
                    nc.vector.reciprocal(recip_sb[:], se_ps[:, 0:1])
                    out_sb = work.tile([BSH, RNN], F32)
                    nc.scalar.activation(out_sb[:, 0:512], res_ps0[:], AF.Copy,
                                         bias=0.0, scale=recip_sb[:, 0:1])
                    nc.sync.dma_start(out=out_d[:, 0:512],
                                      in_=out_sb[:, 0:512])
                    nc.scalar.activation(out_sb[:, 512:1024], res_ps1[:],
                                         AF.Copy,
                                         bias=0.0, scale=recip_sb[:, 0:1])
                    nc.sync.dma_start(out=out_d[:, 512:1024],
                                      in_=out_sb[:, 512:1024])

    nc.compile()
    return nc


def make_in_maps(h, att_feats, p_att_feats, w_h2att, b_h2att, w_alpha,
                 p_fp8=False):
    """Host-side prep: shard over batch, cast to bf16, pre-transpose W/h."""
    PNP = F8NP if p_fp8 else BF
    h = np.asarray(h, dtype=np.float32)
    att_feats = np.asarray(att_feats, dtype=np.float32)
    p_att_feats = np.asarray(p_att_feats, dtype=np.float32)
    w_h2att = np.asarray(w_h2att, dtype=np.float32)
    b_h2att = np.asarray(b_h2att, dtype=np.float32).reshape(1, ATTH)
    w_alpha = np.asarray(w_alpha, dtype=np.float32).reshape(1, ATTH)

    # wT[r, a] chunked over 8 r-chunks of 128: [128, 8*ATTH]
    wT = np.ascontiguousarray(
        w_h2att.T.reshape(8, 128, ATTH).transpose(1, 0, 2).reshape(
            128, 8 * ATTH)).astype(BF)
    bias16 = b_h2att.astype(BF)
    walpha16 = w_alpha.astype(BF)

    in_maps = []
    for c in range(NCORES):
        lo = c * BSH
        hi = lo + BSH
        hT = np.ascontiguousarray(
            h[lo:hi].T.reshape(8, 128, BSH).transpose(1, 0, 2).reshape(
                128, 8 * BSH)).astype(BF)
        in_maps.append({
            "hT": hT,
            "att": np.ascontiguousarray(
                att_feats[lo:hi].reshape(G, RNN)).astype(BF),
            "p_att": np.ascontiguousarray(
                p_att_feats[lo:hi].reshape(G, ATTH)).astype(PNP),
            "wT": wT,
            "b_h2att": bias16,
            "w_alpha": walpha16,
        })
    return in_maps


# production build configuration (kernel() and test.py timing use this)
BUILD_KW = dict(att_q2="scalar", p_fp8=False, score_pat="vav",
                groups7=True, deep_bufs=True)


def kernel(h, att_feats, p_att_feats, w_h2att, b_h2att, w_alpha, b_alpha):
    """Full-input entry point. b_alpha is dropped: softmax is shift-invariant."""
    if "nc" not in _cached:
        _cached["nc"] = build_nc(**BUILD_KW)
    nc = _cached["nc"]

    in_maps = make_in_maps(h, att_feats, p_att_feats, w_h2att, b_h2att,
                           w_alpha, p_fp8=BUILD_KW["p_fp8"])
    res = run_bass_kernel_spmd(nc, in_maps, list(range(NCORES)))
    out = np.concatenate([res.results[c]["out"] for c in range(NCORES)],
                         axis=0)
    return out.astype(np.float32)


# revision 45
# speedup vs baseline: 1.4892x; 1.1506x over previous
"""Trainium2 Bass kernel for additive attention (nn_Attention).

Reference computation (per batch b):
    att_h  = h @ W.T + b_h2att                      [B, ATTH]
    dot    = tanh(p_att_feats + att_h[:, None, :])  [B, S, ATTH]
    scores = dot @ w_alpha[0] (+ b_alpha)           [B, S]
    weight = softmax(scores, axis=1)
    out    = weight @ att_feats                     [B, RNN]

Sharding: data-parallel over batch, 32 batches per core x 8 cores.

All inputs are converted to bf16 on the host (DMA is the roofline) and
W/h are host-pre-transposed. Per-core layout: (batch, S) flattened to
G = 32*196 = 6272 rows = exactly 49 tiles of 128 partitions.

TensorE is kept to the irreducible minimum (att_res/sumexp matmuls) so
the kernel stays fast even when the PE clock is thermally throttled:
  - setup builds zbc[t] = att_h[batch(row)] for every tile row via 49
    one-time bsel matmuls (0/1 batch-selector), copied to SBUF bf16.
  - per iteration, z = p + zbc is ONE DVE add per DMA group, and
    dot = tanh(z) is ONE ScalarE call per group.
  - scores col per tile: either a fused custom-DVE TENSOR_TENSOR_REDUCE
    ('v') or a DVE multiply + ScalarE accumulate copy ('a'), pattern
    chosen to balance DVE vs ScalarE occupancy.
  - e = exp(scores); lhsT[p, b] = e[p] * (batch(p)==b) on DVE (bf16)
  - att_res += lhsT.T @ att_tile ; sumexp += lhsT.T @ ones  (TensorE)
Final: out = att_res * (1/sumexp) fused into the PSUM->SBUF copy.
(b_alpha is dropped: softmax is shift-invariant; |scores| <~ 20 so the
unshifted exp stays in fp32 range.)
"""

import numpy as np
import ml_dtypes

import concourse.bass as bass
import concourse.tile as tile
from concourse import bacc, mybir
from concourse.bass_utils import run_bass_kernel_spmd
from concourse.dve_ops import TENSOR_TENSOR_REDUCE

F32 = mybir.dt.float32
BF16 = mybir.dt.bfloat16
FP8 = mybir.dt.float8e4
AF = mybir.ActivationFunctionType
ALU = mybir.AluOpType
BF = ml_dtypes.bfloat16
F8NP = mybir.dt.np(FP8)

B, S, RNN, ATTH = 256, 196, 1024, 512
NCORES = 8
BSH = B // NCORES            # 32 batches per core
G = BSH * S                  # 6272 rows per core
NT = G // 128                # 49 tiles
assert NT * 128 == G

_cached = {}


def _batch_of_row(g):
    return g // S


def build_nc(repeats=1, att_q2="scalar", p_fp8=False, score_pat="vav",
             ablate=None, att_split="col", groups7=False,
             lhsT_pool=False, prefetch=False, deep_bufs=False,
             p_q="gpsimd"):
    # p_q: queue for the p stream; "sync" reroutes p to the hardware DGE
    #   and moves the att halves to scalar+gpsimd instead
    # lhsT_pool: build lhsT masked-weight columns on GPSIMD instead of DVE
    # prefetch: issue group g+1's DMA loads before group g's compute so
    #   queue-order never blocks the next transfer behind compute deps
    # att_q2: second DMA queue for the att halves; None = single sync queue
    # p_fp8: DMA p_att as fp8e4m3 (halves its bytes; scores lose ~1 digit)
    # score_pat: per-tile scores-reduce engine, indexed by t % len(pat):
    #   'v' = fused custom-DVE op, 'a' = DVE multiply + ScalarE accum
    # ablate="dma": loads + finalize only (timing probe, wrong results)
    # att_split: "col" = halve by feature columns (1KB lines),
    #            "part" = halve by partition range (full 2KB lines)
    # groups7: 7 DMA groups of 7 tiles instead of [4,5,6x6,4]
    PDT = FP8 if p_fp8 else BF16
    GROUPS = [7] * 7 if groups7 else [4, 5, 6, 6, 6, 6, 6, 6, 4]
    assert sum(GROUPS) == NT
    GSTART = [sum(GROUPS[:i]) for i in range(len(GROUPS))]

    def _rep_groups(repeats):
        for r in range(repeats):
            for g in range(len(GROUPS)):
                yield r, g
    nc = bacc.Bacc("TRN2", target_bir_lowering=False, debug=False,
                   enable_asserts=True, num_devices=NCORES)

    hT_d = nc.dram_tensor("hT", [128, 8 * BSH], BF16, kind="ExternalInput")
    att_d = nc.dram_tensor("att", [G, RNN], BF16, kind="ExternalInput")
    p_d = nc.dram_tensor("p_att", [G, ATTH], PDT, kind="ExternalInput")
    wT_d = nc.dram_tensor("wT", [128, 8 * ATTH], BF16, kind="ExternalInput")
    bias_d = nc.dram_tensor("b_h2att", [1, ATTH], BF16, kind="ExternalInput")
    walpha_d = nc.dram_tensor("w_alpha", [1, ATTH], BF16,
                              kind="ExternalInput")
    out_d = nc.dram_tensor("out", [BSH, RNN], F32, kind="ExternalOutput")

    # --- host-side constants, embedded in the NEFF ---
    ones_np = np.ones((128, 128), dtype=np.float32).astype(BF)
    # maskT[p, t, b] = 1 if batch(128t + p) == b
    maskT_np = np.zeros((128, NT, BSH), dtype=np.float32)
    for t in range(NT):
        for p in range(128):
            bb = _batch_of_row(128 * t + p)
            maskT_np[p, t, bb] = 1.0
    # bsel[b, t, p]: one-hot selector, bsel.T @ att_h broadcasts per-row att_h
    bsel_np = np.ascontiguousarray(maskT_np.transpose(2, 1, 0)).astype(BF)
    maskT_np = maskT_np.astype(BF)

    ones_c = nc.inline_tensor(ones_np, "c_ones")
    bsel_c = nc.inline_tensor(bsel_np.reshape(BSH, NT * 128), "c_bsel")
    maskT_c = nc.inline_tensor(maskT_np.reshape(128, NT * BSH), "c_maskT")

    with tile.TileContext(nc) as tc:
        import contextlib
        ctx = contextlib.ExitStack()
        with ctx:
            consts = ctx.enter_context(tc.tile_pool(name="consts", bufs=1))
            work = ctx.enter_context(tc.tile_pool(name="work", bufs=1))
            zbc_pool = ctx.enter_context(tc.tile_pool(name="zbc", bufs=1))
            p_pool = ctx.enter_context(
                tc.tile_pool(name="p_pool", bufs=3 if deep_bufs else 2))
            a_pool = ctx.enter_context(
                tc.tile_pool(name="a_pool", bufs=6 if deep_bufs else 4))
            setup_sb_cm = tc.tile_pool(name="setup_sb", bufs=1)
            setup_sb = setup_sb_cm.__enter__()
            res_pool = ctx.enter_context(
                tc.tile_pool(name="respsum", bufs=1, space="PSUM"))
            setup_ps_cm = tc.tile_pool(name="setupps", bufs=2, space="PSUM")
            setup_ps = setup_ps_cm.__enter__()

            # ---- load constants and small inputs ----
            hT_sb = setup_sb.tile([128, 8 * BSH], BF16)
            nc.gpsimd.dma_start(out=hT_sb[:], in_=hT_d[:])
            wT_sb = setup_sb.tile([128, 8 * ATTH], BF16)
            nc.gpsimd.dma_start(out=wT_sb[:], in_=wT_d[:])
            ones_sb = consts.tile([128, 128], BF16)
            nc.scalar.dma_start(out=ones_sb[:], in_=ones_c[:])
            onesr_sb = consts.tile([128, 2], BF16)
            nc.scalar.dma_start(out=onesr_sb[:], in_=ones_c[:, 0:2])
            bias_sb = setup_sb.tile([1, ATTH], BF16)
            nc.scalar.dma_start(out=bias_sb[:], in_=bias_d[:])
            walpha_sb = setup_sb.tile([1, ATTH], BF16)
            nc.scalar.dma_start(out=walpha_sb[:], in_=walpha_d[:])
            bsel_sb = setup_sb.tile([BSH, NT * 128], BF16)
            nc.scalar.dma_start(out=bsel_sb[:], in_=bsel_c[:])
            maskT_sb = consts.tile([128, NT * BSH], BF16)
            nc.scalar.dma_start(out=maskT_sb[:], in_=maskT_c[:])

            # ---- att_h = h @ W.T + bias via pre-transposed hT/wT ----
            ah_full = setup_ps.tile([128, 1024], F32, tag="zb")
            ah_ps = ah_full[0:BSH, 0:ATTH]
            for rc in range(8):
                nc.tensor.matmul(
                    ah_ps[:],
                    lhsT=hT_sb[:, rc * BSH:(rc + 1) * BSH],
                    rhs=wT_sb[:, rc * ATTH:(rc + 1) * ATTH],
                    start=(rc == 0), stop=False)
            nc.tensor.matmul(
                ah_ps[:], lhsT=ones_sb[0:1, 0:BSH], rhs=bias_sb[0:1, :],
                start=False, stop=True)
            atth_sb = work.tile([BSH, ATTH], BF16)
            nc.vector.tensor_copy(atth_sb[:], ah_ps[:])

            # ---- broadcast w_alpha to all 128 partitions (bf16) ----
            wb_full = setup_ps.tile([128, 1024], F32, tag="zb")
            wb_ps = wb_full[:, 0:ATTH]
            nc.tensor.matmul(wb_ps[:], lhsT=ones_sb[0:1, 0:128],
                             rhs=walpha_sb[0:1, :], start=True, stop=True)
            wb_sb = work.tile([128, ATTH], BF16)
            nc.vector.tensor_copy(wb_sb[:], wb_ps[:])

            # ---- zbc[t][p, :] = att_h[batch(128t+p)], for every tile,
            # built once by TensorE bsel matmuls + copied to SBUF bf16.
            # Stored per DMA group so the per-iteration z-add is one DVE
            # tensor_tensor per group. ----
            zbc_g = {}
            zbc_slice = {}
            for g in range(len(GROUPS)):
                lo, hi = GSTART[g], GSTART[g] + GROUPS[g]
                zt = zbc_pool.tile([128, GROUPS[g] * ATTH], BF16,
                                   tag=f"zbcg{g}", name=f"zbcg{g}")
                zbc_g[g] = zt
                for t in range(lo, hi):
                    zbc_slice[t] = zt[:, (t - lo) * ATTH:(t - lo + 1) * ATTH]
                for i in range(0, GROUPS[g], 2):
                    t0 = lo + i
                    t1 = t0 + 1 if i + 1 < GROUPS[g] else None
                    n_z = 1024 if t1 is not None else 512
                    zb_ps = setup_ps.tile([128, 1024], F32, tag="zb")
                    for j, t in enumerate((t0, t1)):
                        if t is None:
                            continue
                        nc.tensor.matmul(
                            zb_ps[:, j * 512:(j + 1) * 512],
                            lhsT=bsel_sb[:, t * 128:(t + 1) * 128],
                            rhs=atth_sb[:], start=True, stop=True)
                    dst = zt[:, i * ATTH: i * ATTH + n_z]
                    if (i // 2) % 2 == 0:
                        nc.vector.tensor_copy(dst, zb_ps[:, 0:n_z])
                    else:
                        nc.scalar.activation(dst, zb_ps[:, 0:n_z], AF.Copy)
            setup_sb_cm.__exit__(None, None, None)
            setup_ps_cm.__exit__(None, None, None)

            zsb_pool = ctx.enter_context(tc.tile_pool(name="zsb", bufs=2))
            dot_pool = ctx.enter_context(tc.tile_pool(name="dot", bufs=2))
            prod_pool = ctx.enter_context(tc.tile_pool(name="prod", bufs=2))
            small_pool = ctx.enter_context(tc.tile_pool(name="small", bufs=4))

            # ---- persistent accumulators ----
            res_ps0 = res_pool.tile([BSH, 512], F32, tag="res0")
            res_ps1 = res_pool.tile([BSH, 512], F32, tag="res1")
            se_ps = res_pool.tile([BSH, 2], F32, tag="sumexp")

            p_view = p_d[:].rearrange("(t p) e -> p t e", p=128)
            a_view = att_d[:].rearrange("(t p) e -> p t e", p=128)

            p_tiles = {}
            p_gtile = {}
            a_tiles = {}
            GMAX = max(GROUPS)

            def load_p_group(g):
                lo = GSTART[g]
                hi = lo + GROUPS[g]
                # gpsimd (software DGE) DMAs can cast: with p_fp8 the HBM
                # side is fp8e4m3 but SBUF receives bf16, so the DVE z-add
                # keeps its 2x 16-bit mode.
                t_ = p_pool.tile([128, GMAX * ATTH], BF16, tag="pg")
                getattr(nc, p_q).dma_start(
                    out=t_[:, 0:(hi - lo) * ATTH].rearrange(
                        "p (t e) -> p t e", e=ATTH),
                    in_=p_view[:, lo:hi, :])
                p_gtile[g] = t_
                for t in range(lo, hi):
                    p_tiles[t] = t_[:, (t - lo) * ATTH:(t - lo + 1) * ATTH]

            def load_a_group(g):
                lo = GSTART[g]
                hi = lo + GROUPS[g]
                t_ = a_pool.tile([128, GMAX * RNN], BF16, tag="ag")
                # optionally split halves across two DMA queues to stay
                # under the per-queue throughput cap
                view = t_[:, 0:(hi - lo) * RNN].rearrange(
                    "p (t e) -> p t e", e=RNN)
                if att_q2 is None:
                    nc.sync.dma_start(out=view[:], in_=a_view[:, lo:hi, :])
                elif att_split == "tile":
                    # alternate tiles across queues: keeps full 2KB DRAM
                    # lines per (partition, tile) instead of 1KB half-lines
                    qa = nc.sync if p_q == "gpsimd" else nc.scalar
                    qb = getattr(nc, att_q2) if p_q == "gpsimd" \
                        else nc.gpsimd
                    ng_ = hi - lo
                    qa.dma_start(
                        out=view[:, 0:ng_:2, :], in_=a_view[:, lo:hi:2, :])
                    qb.dma_start(
                        out=view[:, 1:ng_:2, :],
                        in_=a_view[:, lo + 1:hi:2, :])
                elif att_split == "part":
                    q2 = getattr(nc, att_q2)
                    nc.sync.dma_start(
                        out=view[0:64], in_=a_view[0:64, lo:hi, :])
                    q2.dma_start(
                        out=view[64:128], in_=a_view[64:128, lo:hi, :])
                else:
                    q2 = getattr(nc, att_q2)
                    nc.sync.dma_start(
                        out=view[:, :, 0:512], in_=a_view[:, lo:hi, 0:512])
                    q2.dma_start(
                        out=view[:, :, 512:1024],
                        in_=a_view[:, lo:hi, 512:1024])
                for t in range(lo, hi):
                    a_tiles[t] = t_[:, (t - lo) * RNN:(t - lo + 1) * RNN]

            # ---- main loop over 49 row-tiles, grouped by DMA group ----
            seq = list(_rep_groups(repeats))
            if prefetch:
                load_p_group(seq[0][1])
                load_a_group(seq[0][1])
            for idx, (_rep, g) in enumerate(seq):
                lo = GSTART[g]
                hi = lo + GROUPS[g]
                ng = hi - lo
                if prefetch:
                    if idx + 1 < len(seq):
                        load_p_group(seq[idx + 1][1])
                        load_a_group(seq[idx + 1][1])
                else:
                    load_p_group(g)
                    load_a_group(g)

                if ablate == "dma":
                    if g == len(GROUPS) - 1:
                        out_sb = work.tile([BSH, RNN], F32)
                        # touch the loaded tiles so DMA isn't dead-code
                        nc.vector.tensor_copy(
                            out_sb[:], a_tiles[lo][0:BSH, :])
                        nc.sync.dma_start(out=out_d[:], in_=out_sb[:])
                    continue

                # z = p + zbc: one DVE add per group; dot = tanh(z): one
                # ScalarE call per group
                z_sb = zsb_pool.tile([128, GMAX * ATTH], BF16, tag="z")
                nc.vector.tensor_tensor(
                    out=z_sb[:, 0:ng * ATTH],
                    in0=p_gtile[g][:, 0:ng * ATTH],
                    in1=zbc_g[g][:], op=ALU.add)
                dot_sb = dot_pool.tile([128, GMAX * ATTH], BF16, tag="dot")
                nc.scalar.activation(dot_sb[:, 0:ng * ATTH],
                                     z_sb[:, 0:ng * ATTH], AF.Tanh)

                scol_g = small_pool.tile([128, GROUPS[g]], F32, tag="scol")
                for t in range(lo, hi):
                    dslice = dot_sb[:, (t - lo) * ATTH:(t - lo + 1) * ATTH]
                    scol = scol_g[:, t - lo: t - lo + 1]
                    prod_sb = prod_pool.tile([128, ATTH], BF16, tag="prod")
                    if score_pat[t % len(score_pat)] == "v":
                        # fused multiply+reduce on DVE
                        nc.vector._custom_dve(
                            TENSOR_TENSOR_REDUCE, out=prod_sb[:],
                            in0=dslice, in1=wb_sb[:], s0=0.0, s1=1.0,
                            imm2=0.0, accum_out=scol)
                    else:
                        # DVE multiply + ScalarE accumulate copy
                        nc.vector.tensor_tensor(
                            out=prod_sb[:], in0=dslice, in1=wb_sb[:],
                            op=ALU.mult)
                        nc.scalar.activation(
                            dslice, prod_sb[:], AF.Copy, bias=0.0,
                            scale=1.0, accum_out=scol)

                # e = exp(scores) for the whole group
                ecol_g = small_pool.tile([128, GROUPS[g]], F32, tag="ecol")
                nc.scalar.activation(ecol_g[:], scol_g[:], AF.Exp)

                for t in range(lo, hi):
                    # masked weight columns: lhsT[p, b] = e[p] * mask[p, b]
                    lhsT_t = small_pool.tile([128, BSH], BF16, tag="lhsT")
                    ts_eng = nc.gpsimd if lhsT_pool else nc.vector
                    ts_eng.tensor_scalar(
                        out=lhsT_t[:], in0=maskT_sb[:, t * BSH:(t + 1) * BSH],
                        scalar1=ecol_g[:, t - lo: t - lo + 1], scalar2=None,
                        op0=ALU.mult)

                    # att_res += lhsT.T @ A ; sumexp += lhsT.T @ 1
                    nc.tensor.matmul(
                        res_ps0[:], lhsT=lhsT_t[:],
                        rhs=a_tiles[t][:, 0:512],
                        start=(t == 0), stop=(t == NT - 1))
                    nc.tensor.matmul(
                        res_ps1[:], lhsT=lhsT_t[:],
                        rhs=a_tiles[t][:, 512:1024],
                        start=(t == 0), stop=(t == NT - 1))
                    nc.tensor.matmul(
                        se_ps[:], lhsT=lhsT_t[:], rhs=onesr_sb[:],
                        start=(t == 0), stop=(t == NT - 1))

                # ---- finalize: out = att_res / sumexp (per repeat so no
                # repeat is dead code in benchmark builds) ----
                if g == len(GROUPS) - 1:
                    recip_sb = work.tile([BSH, 1], F32)
                    nc.vector.reciprocal(recip_sb[:], se_ps[:, 0:1])
                    out_sb = work.tile([BSH, RNN], F32)
                    nc.scalar.activation(out_sb[:, 0:512], res_ps0[:], AF.Copy,
                                         bias=0.0, scale=recip_sb[:, 0:1])
                    nc.sync.dma_start(out=out_d[:, 0:512],
                                      in_=out_sb[:, 0:512])
                    nc.scalar.activation(out_sb[:, 512:1024], res_ps1[:],
                                         AF.Copy,
                                         bias=0.0, scale=recip_sb[:, 0:1])
                    nc.sync.dma_start(out=out_d[:, 512:1024],
                                      in_=out_sb[:, 512:1024])

    nc.compile()
    return nc


def make_in_maps(h, att_feats, p_att_feats, w_h2att, b_h2att, w_alpha,
                 p_fp8=False):
    """Host-side prep: shard over batch, cast to bf16, pre-transpose W/h."""
    PNP = F8NP if p_fp8 else BF
    h = np.asarray(h, dtype=np.float32)
    att_feats = np.asarray(att_feats, dtype=np.float32)
    p_att_feats = np.asarray(p_att_feats, dtype=np.float32)
    w_h2att = np.asarray(w_h2att, dtype=np.float32)
    b_h2att = np.asarray(b_h2att, dtype=np.float32).reshape(1, ATTH)
    w_alpha = np.asarray(w_alpha, dtype=np.float32).reshape(1, ATTH)

    # wT[r, a] chunked over 8 r-chunks of 128: [128, 8*ATTH]
    wT = np.ascontiguousarray(
        w_h2att.T.reshape(8, 128, ATTH).transpose(1, 0, 2).reshape(
            128, 8 * ATTH)).astype(BF)
    bias16 = b_h2att.astype(BF)
    walpha16 = w_alpha.astype(BF)

    in_maps = []
    for c in range(NCORES):
        lo = c * BSH
        hi = lo + BSH
        hT = np.ascontiguousarray(
            h[lo:hi].T.reshape(8, 128, BSH).transpose(1, 0, 2).reshape(
                128, 8 * BSH)).astype(BF)
        in_maps.append({
            "hT": hT,
            "att": np.ascontiguousarray(
                att_feats[lo:hi].reshape(G, RNN)).astype(BF),
            "p_att": np.ascontiguousarray(
                p_att_feats[lo:hi].reshape(G, ATTH)).astype(PNP),
            "wT": wT,
            "b_h2att": bias16,
            "w_alpha": walpha16,
        })
    return in_maps


# production build configuration (kernel() and test.py timing use this)
BUILD_KW = dict(att_q2="scalar", p_fp8=False, score_pat="vav",
                groups7=True, deep_bufs=True, att_split="tile")


def kernel(h, att_feats, p_att_feats, w_h2att, b_h2att, w_alpha, b_alpha):
    """Full-input entry point. b_alpha is dropped: softmax is shift-invariant."""
    if "nc" not in _cached:
        _cached["nc"] = build_nc(**BUILD_KW)
    nc = _cached["nc"]

    in_maps = make_in_maps(h, att_feats, p_att_feats, w_h2att, b_h2att,
                           w_alpha, p_fp8=BUILD_KW["p_fp8"])
    res = run_bass_kernel_spmd(nc, in_maps, list(range(NCORES)))
    out = np.concatenate([res.results[c]["out"] for c in range(NCORES)],
                         axis=0)
    return out.astype(np.float32)


# revision 48
# speedup vs baseline: 1.5642x; 1.0504x over previous
"""Trainium2 Bass kernel for additive attention (nn_Attention).

Reference computation (per batch b):
    att_h  = h @ W.T + b_h2att                      [B, ATTH]
    dot    = tanh(p_att_feats + att_h[:, None, :])  [B, S, ATTH]
    scores = dot @ w_alpha[0] (+ b_alpha)           [B, S]
    weight = softmax(scores, axis=1)
    out    = weight @ att_feats                     [B, RNN]

Sharding: data-parallel over batch, 32 batches per core x 8 cores.

All inputs are converted to bf16 on the host (DMA is the roofline) and
W/h are host-pre-transposed. Per-core layout: (batch, S) flattened to
G = 32*196 = 6272 rows = exactly 49 tiles of 128 partitions.

TensorE is kept to the irreducible minimum (att_res/sumexp matmuls) so
the kernel stays fast even when the PE clock is thermally throttled:
  - setup builds zbc[t] = att_h[batch(row)] for every tile row via 49
    one-time bsel matmuls (0/1 batch-selector), copied to SBUF bf16.
  - per iteration, z = p + zbc is ONE DVE add per DMA group, and
    dot = tanh(z) is ONE ScalarE call per group.
  - scores col per tile: either a fused custom-DVE TENSOR_TENSOR_REDUCE
    ('v') or a DVE multiply + ScalarE accumulate copy ('a'), pattern
    chosen to balance DVE vs ScalarE occupancy.
  - e = exp(scores); lhsT[p, b] = e[p] * (batch(p)==b) on DVE (bf16)
  - att_res += lhsT.T @ att_tile ; sumexp += lhsT.T @ ones  (TensorE)
Final: out = att_res * (1/sumexp) fused into the PSUM->SBUF copy.
(b_alpha is dropped: softmax is shift-invariant; |scores| <~ 20 so the
unshifted exp stays in fp32 range.)
"""

import numpy as np
import ml_dtypes

import concourse.bass as bass
import concourse.tile as tile
from concourse import bacc, mybir
from concourse.bass_utils import run_bass_kernel_spmd
from concourse.dve_ops import TENSOR_TENSOR_REDUCE

F32 = mybir.dt.float32
BF16 = mybir.dt.bfloat16
FP8 = mybir.dt.float8e4
AF = mybir.ActivationFunctionType
ALU = mybir.AluOpType
BF = ml_dtypes.bfloat16
F8NP = mybir.dt.np(FP8)

B, S, RNN, ATTH = 256, 196, 1024, 512
NCORES = 8
BSH = B // NCORES            # 32 batches per core
G = BSH * S                  # 6272 rows per core
NT = G // 128                # 49 tiles
assert NT * 128 == G

_cached = {}


def _batch_of_row(g):
    return g // S


def build_nc(repeats=1, att_q2="scalar", p_fp8=False, score_pat="vav",
             ablate=None, att_split="col", groups7=False,
             lhsT_pool=False, prefetch=False, deep_bufs=False,
             p_q="gpsimd", rings=False):
    # rings: trade one att prefetch buffer for deeper compute rings
    #   (a_pool 5, dot/zsb 3) to destall the tanh->scores pipeline
    # p_q: queue for the p stream; "sync" reroutes p to the hardware DGE
    #   and moves the att halves to scalar+gpsimd instead
    # lhsT_pool: build lhsT masked-weight columns on GPSIMD instead of DVE
    # prefetch: issue group g+1's DMA loads before group g's compute so
    #   queue-order never blocks the next transfer behind compute deps
    # att_q2: second DMA queue for the att halves; None = single sync queue
    # p_fp8: DMA p_att as fp8e4m3 (halves its bytes; scores lose ~1 digit)
    # score_pat: per-tile scores-reduce engine, indexed by t % len(pat):
    #   'v' = fused custom-DVE op, 'a' = DVE multiply + ScalarE accum
    # ablate="dma": loads + finalize only (timing probe, wrong results)
    # att_split: "col" = halve by feature columns (1KB lines),
    #            "part" = halve by partition range (full 2KB lines)
    # groups7: 7 DMA groups of 7 tiles instead of [4,5,6x6,4]
    PDT = FP8 if p_fp8 else BF16
    GROUPS = [7] * 7 if groups7 else [4, 5, 6, 6, 6, 6, 6, 6, 4]
    assert sum(GROUPS) == NT
    GSTART = [sum(GROUPS[:i]) for i in range(len(GROUPS))]

    def _rep_groups(repeats):
        for r in range(repeats):
            for g in range(len(GROUPS)):
                yield r, g
    nc = bacc.Bacc("TRN2", target_bir_lowering=False, debug=False,
                   enable_asserts=True, num_devices=NCORES)

    hT_d = nc.dram_tensor("hT", [128, 8 * BSH], BF16, kind="ExternalInput")
    att_d = nc.dram_tensor("att", [G, RNN], BF16, kind="ExternalInput")
    p_d = nc.dram_tensor("p_att", [G, ATTH], PDT, kind="ExternalInput")
    wT_d = nc.dram_tensor("wT", [128, 8 * ATTH], BF16, kind="ExternalInput")
    bias_d = nc.dram_tensor("b_h2att", [1, ATTH], BF16, kind="ExternalInput")
    walpha_d = nc.dram_tensor("w_alpha", [1, ATTH], BF16,
                              kind="ExternalInput")
    out_d = nc.dram_tensor("out", [BSH, RNN], F32, kind="ExternalOutput")

    # --- host-side constants, embedded in the NEFF ---
    ones_np = np.ones((128, 128), dtype=np.float32).astype(BF)
    # maskT[p, t, b] = 1 if batch(128t + p) == b
    maskT_np = np.zeros((128, NT, BSH), dtype=np.float32)
    for t in range(NT):
        for p in range(128):
            bb = _batch_of_row(128 * t + p)
            maskT_np[p, t, bb] = 1.0
    # bsel[b, t, p]: one-hot selector, bsel.T @ att_h broadcasts per-row att_h
    bsel_np = np.ascontiguousarray(maskT_np.transpose(2, 1, 0)).astype(BF)
    maskT_np = maskT_np.astype(BF)

    ones_c = nc.inline_tensor(ones_np, "c_ones")
    bsel_c = nc.inline_tensor(bsel_np.reshape(BSH, NT * 128), "c_bsel")
    maskT_c = nc.inline_tensor(maskT_np.reshape(128, NT * BSH), "c_maskT")

    with tile.TileContext(nc) as tc:
        import contextlib
        ctx = contextlib.ExitStack()
        with ctx:
            consts = ctx.enter_context(tc.tile_pool(name="consts", bufs=1))
            work = ctx.enter_context(tc.tile_pool(name="work", bufs=1))
            zbc_pool = ctx.enter_context(tc.tile_pool(name="zbc", bufs=1))
            p_pool = ctx.enter_context(
                tc.tile_pool(name="p_pool", bufs=3 if deep_bufs else 2))
            a_pool = ctx.enter_context(
                tc.tile_pool(
                    name="a_pool",
                    bufs=(5 if rings else 6) if deep_bufs else 4))
            setup_sb_cm = tc.tile_pool(name="setup_sb", bufs=1)
            setup_sb = setup_sb_cm.__enter__()
            res_pool = ctx.enter_context(
                tc.tile_pool(name="respsum", bufs=1, space="PSUM"))
            setup_ps_cm = tc.tile_pool(name="setupps", bufs=2, space="PSUM")
            setup_ps = setup_ps_cm.__enter__()

            # ---- load constants and small inputs ----
            hT_sb = setup_sb.tile([128, 8 * BSH], BF16)
            nc.gpsimd.dma_start(out=hT_sb[:], in_=hT_d[:])
            wT_sb = setup_sb.tile([128, 8 * ATTH], BF16)
            nc.gpsimd.dma_start(out=wT_sb[:], in_=wT_d[:])
            ones_sb = consts.tile([128, 128], BF16)
            nc.scalar.dma_start(out=ones_sb[:], in_=ones_c[:])
            onesr_sb = consts.tile([128, 2], BF16)
            nc.scalar.dma_start(out=onesr_sb[:], in_=ones_c[:, 0:2])
            bias_sb = setup_sb.tile([1, ATTH], BF16)
            nc.scalar.dma_start(out=bias_sb[:], in_=bias_d[:])
            walpha_sb = setup_sb.tile([1, ATTH], BF16)
            nc.scalar.dma_start(out=walpha_sb[:], in_=walpha_d[:])
            bsel_sb = setup_sb.tile([BSH, NT * 128], BF16)
            nc.scalar.dma_start(out=bsel_sb[:], in_=bsel_c[:])
            maskT_sb = consts.tile([128, NT * BSH], BF16)
            nc.scalar.dma_start(out=maskT_sb[:], in_=maskT_c[:])

            # ---- att_h = h @ W.T + bias via pre-transposed hT/wT ----
            ah_full = setup_ps.tile([128, 1024], F32, tag="zb")
            ah_ps = ah_full[0:BSH, 0:ATTH]
            for rc in range(8):
                nc.tensor.matmul(
                    ah_ps[:],
                    lhsT=hT_sb[:, rc * BSH:(rc + 1) * BSH],
                    rhs=wT_sb[:, rc * ATTH:(rc + 1) * ATTH],
                    start=(rc == 0), stop=False)
            nc.tensor.matmul(
                ah_ps[:], lhsT=ones_sb[0:1, 0:BSH], rhs=bias_sb[0:1, :],
                start=False, stop=True)
            atth_sb = work.tile([BSH, ATTH], BF16)
            nc.vector.tensor_copy(atth_sb[:], ah_ps[:])

            # ---- broadcast w_alpha to all 128 partitions (bf16) ----
            wb_full = setup_ps.tile([128, 1024], F32, tag="zb")
            wb_ps = wb_full[:, 0:ATTH]
            nc.tensor.matmul(wb_ps[:], lhsT=ones_sb[0:1, 0:128],
                             rhs=walpha_sb[0:1, :], start=True, stop=True)
            wb_sb = work.tile([128, ATTH], BF16)
            nc.vector.tensor_copy(wb_sb[:], wb_ps[:])

            # ---- zbc[t][p, :] = att_h[batch(128t+p)], for every tile,
            # built once by TensorE bsel matmuls + copied to SBUF bf16.
            # Stored per DMA group so the per-iteration z-add is one DVE
            # tensor_tensor per group. ----
            zbc_g = {}
            zbc_slice = {}
            for g in range(len(GROUPS)):
                lo, hi = GSTART[g], GSTART[g] + GROUPS[g]
                zt = zbc_pool.tile([128, GROUPS[g] * ATTH], BF16,
                                   tag=f"zbcg{g}", name=f"zbcg{g}")
                zbc_g[g] = zt
                for t in range(lo, hi):
                    zbc_slice[t] = zt[:, (t - lo) * ATTH:(t - lo + 1) * ATTH]
                for i in range(0, GROUPS[g], 2):
                    t0 = lo + i
                    t1 = t0 + 1 if i + 1 < GROUPS[g] else None
                    n_z = 1024 if t1 is not None else 512
                    zb_ps = setup_ps.tile([128, 1024], F32, tag="zb")
                    for j, t in enumerate((t0, t1)):
                        if t is None:
                            continue
                        nc.tensor.matmul(
                            zb_ps[:, j * 512:(j + 1) * 512],
                            lhsT=bsel_sb[:, t * 128:(t + 1) * 128],
                            rhs=atth_sb[:], start=True, stop=True)
                    dst = zt[:, i * ATTH: i * ATTH + n_z]
                    if (i // 2) % 2 == 0:
                        nc.vector.tensor_copy(dst, zb_ps[:, 0:n_z])
                    else:
                        nc.scalar.activation(dst, zb_ps[:, 0:n_z], AF.Copy)
            setup_sb_cm.__exit__(None, None, None)
            setup_ps_cm.__exit__(None, None, None)

            zsb_pool = ctx.enter_context(
                tc.tile_pool(name="zsb", bufs=3 if rings else 2))
            dot_pool = ctx.enter_context(
                tc.tile_pool(name="dot", bufs=3 if rings else 2))
            prod_pool = ctx.enter_context(tc.tile_pool(name="prod", bufs=2))
            small_pool = ctx.enter_context(tc.tile_pool(name="small", bufs=4))

            # ---- persistent accumulators ----
            res_ps0 = res_pool.tile([BSH, 512], F32, tag="res0")
            res_ps1 = res_pool.tile([BSH, 512], F32, tag="res1")
            se_ps = res_pool.tile([BSH, 2], F32, tag="sumexp")

            p_view = p_d[:].rearrange("(t p) e -> p t e", p=128)
            a_view = att_d[:].rearrange("(t p) e -> p t e", p=128)

            p_tiles = {}
            p_gtile = {}
            a_tiles = {}
            GMAX = max(GROUPS)

            def load_p_group(g):
                lo = GSTART[g]
                hi = lo + GROUPS[g]
                # gpsimd (software DGE) DMAs can cast: with p_fp8 the HBM
                # side is fp8e4m3 but SBUF receives bf16, so the DVE z-add
                # keeps its 2x 16-bit mode.
                t_ = p_pool.tile([128, GMAX * ATTH], BF16, tag="pg")
                getattr(nc, p_q).dma_start(
                    out=t_[:, 0:(hi - lo) * ATTH].rearrange(
                        "p (t e) -> p t e", e=ATTH),
                    in_=p_view[:, lo:hi, :])
                p_gtile[g] = t_
                for t in range(lo, hi):
                    p_tiles[t] = t_[:, (t - lo) * ATTH:(t - lo + 1) * ATTH]

            def load_a_group(g):
                lo = GSTART[g]
                hi = lo + GROUPS[g]
                t_ = a_pool.tile([128, GMAX * RNN], BF16, tag="ag")
                # optionally split halves across two DMA queues to stay
                # under the per-queue throughput cap
                view = t_[:, 0:(hi - lo) * RNN].rearrange(
                    "p (t e) -> p t e", e=RNN)
                if att_q2 is None:
                    nc.sync.dma_start(out=view[:], in_=a_view[:, lo:hi, :])
                elif att_split == "tile":
                    # alternate tiles across queues: keeps full 2KB DRAM
                    # lines per (partition, tile) instead of 1KB half-lines
                    qa = nc.sync if p_q == "gpsimd" else nc.scalar
                    qb = getattr(nc, att_q2) if p_q == "gpsimd" \
                        else nc.gpsimd
                    ng_ = hi - lo
                    qa.dma_start(
                        out=view[:, 0:ng_:2, :], in_=a_view[:, lo:hi:2, :])
                    qb.dma_start(
                        out=view[:, 1:ng_:2, :],
                        in_=a_view[:, lo + 1:hi:2, :])
                elif att_split == "part":
                    q2 = getattr(nc, att_q2)
                    nc.sync.dma_start(
                        out=view[0:64], in_=a_view[0:64, lo:hi, :])
                    q2.dma_start(
                        out=view[64:128], in_=a_view[64:128, lo:hi, :])
                else:
                    q2 = getattr(nc, att_q2)
                    nc.sync.dma_start(
                        out=view[:, :, 0:512], in_=a_view[:, lo:hi, 0:512])
                    q2.dma_start(
                        out=view[:, :, 512:1024],
                        in_=a_view[:, lo:hi, 512:1024])
                for t in range(lo, hi):
                    a_tiles[t] = t_[:, (t - lo) * RNN:(t - lo + 1) * RNN]

            # ---- main loop over 49 row-tiles, grouped by DMA group ----
            seq = list(_rep_groups(repeats))
            if prefetch:
                load_p_group(seq[0][1])
                load_a_group(seq[0][1])
            for idx, (_rep, g) in enumerate(seq):
                lo = GSTART[g]
                hi = lo + GROUPS[g]
                ng = hi - lo
                if prefetch:
                    if idx + 1 < len(seq):
                        load_p_group(seq[idx + 1][1])
                        load_a_group(seq[idx + 1][1])
                else:
                    load_p_group(g)
                    load_a_group(g)

                if ablate == "dma":
                    if g == len(GROUPS) - 1:
                        out_sb = work.tile([BSH, RNN], F32)
                        # touch the loaded tiles so DMA isn't dead-code
                        nc.vector.tensor_copy(
                            out_sb[:], a_tiles[lo][0:BSH, :])
                        nc.sync.dma_start(out=out_d[:], in_=out_sb[:])
                    continue

                # z = p + zbc: one DVE add per group; dot = tanh(z): one
                # ScalarE call per group
                z_sb = zsb_pool.tile([128, GMAX * ATTH], BF16, tag="z")
                nc.vector.tensor_tensor(
                    out=z_sb[:, 0:ng * ATTH],
                    in0=p_gtile[g][:, 0:ng * ATTH],
                    in1=zbc_g[g][:], op=ALU.add)
                dot_sb = dot_pool.tile([128, GMAX * ATTH], BF16, tag="dot")
                nc.scalar.activation(dot_sb[:, 0:ng * ATTH],
                                     z_sb[:, 0:ng * ATTH], AF.Tanh)

                scol_g = small_pool.tile([128, GROUPS[g]], F32, tag="scol")
                for t in range(lo, hi):
                    dslice = dot_sb[:, (t - lo) * ATTH:(t - lo + 1) * ATTH]
                    scol = scol_g[:, t - lo: t - lo + 1]
                    prod_sb = prod_pool.tile([128, ATTH], BF16, tag="prod")
                    if score_pat[t % len(score_pat)] == "v":
                        # fused multiply+reduce on DVE
                        nc.vector._custom_dve(
                            TENSOR_TENSOR_REDUCE, out=prod_sb[:],
                            in0=dslice, in1=wb_sb[:], s0=0.0, s1=1.0,
                            imm2=0.0, accum_out=scol)
                    else:
                        # DVE multiply + ScalarE accumulate copy
                        nc.vector.tensor_tensor(
                            out=prod_sb[:], in0=dslice, in1=wb_sb[:],
                            op=ALU.mult)
                        nc.scalar.activation(
                            dslice, prod_sb[:], AF.Copy, bias=0.0,
                            scale=1.0, accum_out=scol)

                # e = exp(scores) for the whole group
                ecol_g = small_pool.tile([128, GROUPS[g]], F32, tag="ecol")
                nc.scalar.activation(ecol_g[:], scol_g[:], AF.Exp)

                for t in range(lo, hi):
                    # masked weight columns: lhsT[p, b] = e[p] * mask[p, b]
                    lhsT_t = small_pool.tile([128, BSH], BF16, tag="lhsT")
                    ts_eng = nc.gpsimd if lhsT_pool else nc.vector
                    ts_eng.tensor_scalar(
                        out=lhsT_t[:], in0=maskT_sb[:, t * BSH:(t + 1) * BSH],
                        scalar1=ecol_g[:, t - lo: t - lo + 1], scalar2=None,
                        op0=ALU.mult)

                    # att_res += lhsT.T @ A ; sumexp += lhsT.T @ 1
                    nc.tensor.matmul(
                        res_ps0[:], lhsT=lhsT_t[:],
                        rhs=a_tiles[t][:, 0:512],
                        start=(t == 0), stop=(t == NT - 1))
                    nc.tensor.matmul(
                        res_ps1[:], lhsT=lhsT_t[:],
                        rhs=a_tiles[t][:, 512:1024],
                        start=(t == 0), stop=(t == NT - 1))
                    nc.tensor.matmul(
                        se_ps[:], lhsT=lhsT_t[:], rhs=onesr_sb[:],
                        start=(t == 0), stop=(t == NT - 1))

                # ---- finalize: out = att_res / sumexp (per repeat so no
                # repeat is dead code in benchmark builds) ----
                if g == len(GROUPS) - 1:
                    recip_sb = work.tile([BSH, 1], F32)
                    nc.vector.reciprocal(recip_sb[:], se_ps[:, 0:1])
                    out_sb = work.tile([BSH, RNN], F32)
                    nc.scalar.activation(out_sb[:, 0:512], res_ps0[:], AF.Copy,
                                         bias=0.0, scale=recip_sb[:, 0:1])
                    nc.sync.dma_start(out=out_d[:, 0:512],
                                      in_=out_sb[:, 0:512])
                    nc.scalar.activation(out_sb[:, 512:1024], res_ps1[:],
                                         AF.Copy,
                                         bias=0.0, scale=recip_sb[:, 0:1])
                    nc.sync.dma_start(out=out_d[:, 512:1024],
                                      in_=out_sb[:, 512:1024])

    nc.compile()
    return nc


def make_in_maps(h, att_feats, p_att_feats, w_h2att, b_h2att, w_alpha,
                 p_fp8=False):
    """Host-side prep: shard over batch, cast to bf16, pre-transpose W/h."""
    PNP = F8NP if p_fp8 else BF
    h = np.asarray(h, dtype=np.float32)
    att_feats = np.asarray(att_feats, dtype=np.float32)
    p_att_feats = np.asarray(p_att_feats, dtype=np.float32)
    w_h2att = np.asarray(w_h2att, dtype=np.float32)
    b_h2att = np.asarray(b_h2att, dtype=np.float32).reshape(1, ATTH)
    w_alpha = np.asarray(w_alpha, dtype=np.float32).reshape(1, ATTH)

    # wT[r, a] chunked over 8 r-chunks of 128: [128, 8*ATTH]
    wT = np.ascontiguousarray(
        w_h2att.T.reshape(8, 128, ATTH).transpose(1, 0, 2).reshape(
            128, 8 * ATTH)).astype(BF)
    bias16 = b_h2att.astype(BF)
    walpha16 = w_alpha.astype(BF)

    in_maps = []
    for c in range(NCORES):
        lo = c * BSH
        hi = lo + BSH
        hT = np.ascontiguousarray(
            h[lo:hi].T.reshape(8, 128, BSH).transpose(1, 0, 2).reshape(
                128, 8 * BSH)).astype(BF)
        in_maps.append({
            "hT": hT,
            "att": np.ascontiguousarray(
                att_feats[lo:hi].reshape(G, RNN)).astype(BF),
            "p_att": np.ascontiguousarray(
                p_att_feats[lo:hi].reshape(G, ATTH)).astype(PNP),
            "wT": wT,
            "b_h2att": bias16,
            "w_alpha": walpha16,
        })
    return in_maps


# production build configuration (kernel() and test.py timing use this)
BUILD_KW = dict(att_q2="scalar", p_fp8=False, score_pat="vav",
                groups7=True, deep_bufs=True, att_split="tile")


def kernel(h, att_feats, p_att_feats, w_h2att, b_h2att, w_alpha, b_alpha):
    """Full-input entry point. b_alpha is dropped: softmax is shift-invariant."""
    if "nc" not in _cached:
        _cached["nc"] = build_nc(**BUILD_KW)
    nc = _cached["nc"]

    in_maps = make_in_maps(h, att_feats, p_att_feats, w_h2att, b_h2att,
                           w_alpha, p_fp8=BUILD_KW["p_fp8"])
    res = run_bass_kernel_spmd(nc, in_maps, list(range(NCORES)))
    out = np.concatenate([res.results[c]["out"] for c in range(NCORES)],
                         axis=0)
    return out.astype(np.float32)
